# revision 28
# baseline (speedup 1.0000x reference)
"""Trainium2 Bass kernel for multi-head attention with adaptive span masking.

Computation (per the nn.Module):
    q = (query @ Wq.T) split into B*H rows of size d=64
    attn = softmax((key . q + q @ key_pe) / sqrt(d))
    attn = renormalize(attn * adaptive_span_mask)
    out = (attn . value) merged heads @ Wo.T

Key optimization: the adaptive-span mask is exactly zero for positions
m <= (M-1) - RAMP - span[h]*M, so those key/value rows contribute
nothing to the output (their only coupling is the 1e-8*sum(softmax)
term in the renormalization denominator, which perturbs the result by
~5e-6 relative). Each head therefore only reads the tail [mstart_h, M)
of key/value, cutting HBM traffic by ~2x. mstart_h is computed on the
host from the actual span input and baked into the compiled kernel.

Sharding: batch-parallel across 8 cores. Core c gets batches [4c, 4c+4)
(all 8 heads) = rows [32c, 32c+32) of key/value; Wq/Wo/key_pe/span are
replicated. Each core produces its own [4, 512] output block; the host
concatenates. No collectives needed.

Engine split per (batch, head) row:
  - key AND value loaded via gpsimd (SWDGE) DMA with inline f32->bf16
    cast: HBM reads stay f32 (unavoidable) but no engine time is spent
    casting and SBUF footprint halves
  - QK dot on DVE: bf16 multiply + reduce over d
  - positional scores precomputed per head on PE (key_pe stationary)
  - exp on ACT (with fused sum); mask-mult + sum fused in one DVE
    tensor_tensor_reduce; PV accumulation on PE in bf16
  - prefetch is software-pipelined two heads ahead so the DMA queues
    never drain; masks/iotas are computed in setup so the gpsimd queue
    carries only DMA work in the main loop
"""

import math
import os
import sys

import numpy as np

for _p in ("/opt/trn_rl_repo", "/root/.axon_site/_ro/trn_rl_repo"):
    if os.path.isdir(_p) and _p not in sys.path:
        sys.path.insert(0, _p)

import concourse.bass as bass
import concourse.bacc as bacc
import concourse.mybir as mybir
from concourse.bass import ts
from concourse.masks import make_identity
from concourse.tile import TileContext

F32 = mybir.dt.float32
BF16 = mybir.dt.bfloat16

# Problem constants (hardcoded per contest contract)
NHEADS = 8
HEAD_DIM = 64
HID = NHEADS * HEAD_DIM  # 512
B = 32
M = 8192
RAMP = 32.0

N_CORES = 8
BPC = B // N_CORES        # 4 batches per core
NPC = BPC * NHEADS        # 32 (b,h) rows per core

# tensor_tensor_reduce faults the runtime on this deployment; keep the
# two-op mul+reduce path (opt back in with K_TTR=1 to re-test)
USE_TTR = os.environ.get("K_TTR", "") != ""
USE_CAST_DMA = os.environ.get("K_NO_CAST", "") == ""

_CACHE = {}


def compute_mstarts(span: np.ndarray) -> tuple:
    """First key/value position with nonzero mask, per head, aligned
    down to a multiple of 128 (the SBUF partition count).

    mask[h, m] = clip((m - (M-1) + span[h]*M)/RAMP + 1, 0, 1) is zero
    iff m <= (M-1) - RAMP - span[h]*M.
    """
    s = np.asarray(span, np.float64).reshape(-1)
    last_zero = np.floor((M - 1) - RAMP - s * M).astype(np.int64)
    mstart = np.clip(last_zero, 0, M - 128)
    mstart = (mstart // 128) * 128
    return tuple(int(x) for x in mstart)


def build_nc(mstarts):
    nc = bacc.Bacc(None, target_bir_lowering=False)
    AF = mybir.ActivationFunctionType
    ALU = mybir.AluOpType

    mohs = [(M - mstarts[h]) // 128 for h in range(NHEADS)]
    maxmo = max(mohs)
    # big heads first (their DMA overlaps setup), smallest last (short tail)
    order = sorted(range(NHEADS), key=lambda h: -mohs[h])

    q_d = nc.dram_tensor("query", [BPC, HID], F32, kind="ExternalInput")
    k_d = nc.dram_tensor("key", [NPC, M, HEAD_DIM], F32, kind="ExternalInput")
    v_d = nc.dram_tensor("value", [NPC, M, HEAD_DIM], F32, kind="ExternalInput")
    wq_d = nc.dram_tensor("Wq", [HID, HID], F32, kind="ExternalInput")
    wo_d = nc.dram_tensor("Wo", [HID, HID], F32, kind="ExternalInput")
    kpe_d = nc.dram_tensor("key_pe", [HEAD_DIM, M], F32, kind="ExternalInput")
    span_d = nc.dram_tensor("span", [NHEADS, 1], F32, kind="ExternalInput")
    out_d = nc.dram_tensor("out", [BPC, HID], F32, kind="ExternalOutput")

    with TileContext(nc) as tc:
        with (
            tc.tile_pool(name="persist", bufs=1) as persist,
            # main-loop pools created BEFORE setup pools so the kv DMAs get
            # SBUF ranges disjoint from setup tiles (no WAR dep -> kv loads
            # start at t=0, overlapping the whole setup phase)
            tc.tile_pool(name="kv", bufs=8) as kv_pool,
            tc.tile_pool(name="sc", bufs=3) as sc_pool,
            tc.tile_pool(name="fin", bufs=1) as fin_pool,
            tc.tile_pool(name="ps_pos", bufs=2, space="PSUM") as ps_pos_pool,
            tc.tile_pool(name="ps_s", bufs=1, space="PSUM") as ps_s_pool,
            tc.tile_pool(name="ps_o", bufs=2, space="PSUM") as ps_o_pool,
        ):
            identity = persist.tile([128, 128], F32, tag="identity")
            make_identity(nc, identity[:])
            ones_row = persist.tile([1, 128], F32, tag="ones_row")
            nc.vector.memset(ones_row[:], 1.0)
            ones_col = persist.tile([128, 1], F32, tag="ones_col")
            nc.vector.memset(ones_col[:], 1.0)

            woT = [persist.tile([128, HID], F32, name=f"woT{j}", tag=f"woT{j}") for j in range(4)]
            q_sb = persist.tile([BPC, HID], F32, tag="q_sb")
            qts = persist.tile([HEAD_DIM, NHEADS, BPC], F32, tag="qts")
            qts_bf = persist.tile([HEAD_DIM, NHEADS, BPC], BF16, tag="qts_bf")
            qrep_bf = persist.tile([128, BPC, HID], BF16, tag="qrep_bf")
            kpe_sb = persist.tile([HEAD_DIM, M], BF16, tag="kpe_sb")
            span_b = persist.tile([128, NHEADS], F32, tag="span_b")
            span_row = persist.tile([1, NHEADS], F32, tag="span_row")
            pos_sb = [
                persist.tile([128, mohs[h], BPC], F32, name=f"pos{h}", tag=f"pos{h}")
                for h in range(NHEADS)
            ]
            masks = [
                persist.tile([128, mohs[h]], F32, name=f"mask{h}", tag=f"mask{h}")
                for h in range(NHEADS)
            ]
            lm8 = [
                persist.tile([128, mohs[h]], F32, name=f"lm8_{h}", tag=f"lm8_{h}")
                for h in range(NHEADS)
            ]
            ao_sb = persist.tile([1, BPC, HID], F32, tag="ao_sb")

            # key_pe tail (only columns any head can touch), cast to bf16
            # in-flight; first instruction on the gpsimd queue
            mstart_min = min(mstarts)
            nc.gpsimd.dma_start(
                out=kpe_sb[:, mstart_min:M], in_=kpe_d[:, mstart_min:M]
            )

            # ---- span bias + adaptive-span masks, computed FIRST so the
            # row pipeline is never gated on them (they feed every row's
            # mask multiply)
            nc.sync.dma_start(out=span_row[:], in_=span_d[:].rearrange("h o -> o h"))
            span_b2 = persist.tile([128, NHEADS], F32, tag="span_b2")
            with tc.tile_pool(name="ps_span", bufs=1, space="PSUM") as ps_span:
                ps_sp = ps_span.tile([128, NHEADS], F32, tag="ps_sp")
                nc.tensor.matmul(
                    ps_sp[:], ones_row[:], span_row[:], start=True, stop=True
                )
                # span_b[p, h] = span[h]*M/RAMP - (M-1)/RAMP + 1;
                # span_b2 folds in the per-head mstart/RAMP offset
                bias_const = float(-(M - 1) / RAMP + 1.0)
                nc.scalar.activation(
                    out=span_b[:], in_=ps_sp[:], func=AF.Copy,
                    scale=float(M / RAMP), bias=bias_const,
                )
            for h in range(NHEADS):
                nc.scalar.activation(
                    out=span_b2[:, h : h + 1], in_=span_b[:, h : h + 1],
                    func=AF.Copy, bias=float(mstarts[h] / RAMP),
                )
            # masks[h] = clip(..., 1e-30, 1); lm8[h] = 8*ln(masks[h]).
            # The mask multiply is folded into the exp: w = exp(s/8 + ln m)
            # = exp((s + 8 ln m) / 8), with 8*ln(m) pre-added into the
            # positional-score tile once per head. The 1e-30 floor keeps
            # ln() finite; exp then underflows those positions to exactly 0.
            for h in range(NHEADS):
                mo_h = mohs[h]
                m_f = sc_pool.tile([128, maxmo], F32, tag="m_f", bufs=4)
                nc.gpsimd.iota(
                    out=m_f[:, 0:mo_h], pattern=[[1, mo_h]], base=0,
                    channel_multiplier=mo_h,
                    allow_small_or_imprecise_dtypes=True,
                )
                nc.scalar.activation(
                    out=masks[h][:], in_=m_f[:, 0:mo_h], func=AF.Identity,
                    scale=float(1.0 / RAMP), bias=span_b2[:, h : h + 1],
                )
                nc.vector.tensor_scalar(
                    out=masks[h][:], in0=masks[h][:],
                    scalar1=1e-30, scalar2=1.0,
                    op0=ALU.max, op1=ALU.min,
                )
                nc.scalar.activation(
                    out=lm8[h][:], in_=masks[h][:], func=AF.Ln,
                )
                nc.vector.tensor_scalar(
                    out=lm8[h][:], in0=lm8[h][:],
                    scalar1=8.0, scalar2=0.0,
                    op0=ALU.mult, op1=ALU.add,
                )

            # K/V prefetch for one head (4 batch rows), f32->bf16 in-flight
            kv_tiles = {}

            def emit_prefetch(h):
                mo_h = mohs[h]
                mst = mstarts[h]
                tiles = []
                for b in range(BPC):
                    i = b * NHEADS + h
                    if USE_CAST_DMA:
                        kt = kv_pool.tile([128, maxmo, HEAD_DIM], BF16, tag="kt")
                        nc.gpsimd.dma_start(
                            out=kt[:, 0:mo_h, :],
                            in_=k_d[i, mst:M, :].rearrange("(p mo) d -> p mo d", p=128),
                        )
                        vt = kv_pool.tile([128, maxmo, HEAD_DIM], BF16, tag="vt")
                        nc.gpsimd.dma_start(
                            out=vt[:, 0:mo_h, :],
                            in_=v_d[i, mst:M, :].rearrange("(p mo) d -> p mo d", p=128),
                        )
                    else:
                        kt = kv_pool.tile([128, maxmo, HEAD_DIM], F32, tag="kt", bufs=4)
                        nc.sync.dma_start(
                            out=kt[:, 0:mo_h, :],
                            in_=k_d[i, mst:M, :].rearrange("(p mo) d -> p mo d", p=128),
                        )
                        vt = kv_pool.tile([128, maxmo, HEAD_DIM], F32, tag="vt", bufs=4)
                        nc.scalar.dma_start(
                            out=vt[:, 0:mo_h, :],
                            in_=v_d[i, mst:M, :].rearrange("(p mo) d -> p mo d", p=128),
                        )
                    tiles.append((kt, vt))
                kv_tiles[h] = tiles

            emit_prefetch(order[0])
            emit_prefetch(order[1])

            # ---------------- setup phase A: weight transposes + q ----------
            with (
                tc.tile_pool(name="setupA", bufs=1) as sa,
                tc.tile_pool(name="psA", bufs=2, space="PSUM") as psA,
            ):
                wqT = [sa.tile([128, HID], F32, name=f"wqT{j}", tag=f"wqT{j}") for j in range(4)]
                wq_sb = [sa.tile([128, HID], F32, name=f"wq_sb{i}", tag="wq_sb", bufs=2) for i in range(4)]
                for i in range(4):
                    nc.sync.dma_start(out=wq_sb[i][:], in_=wq_d[ts(i, 128), :])
                for io in range(4):
                    for jo in range(4):
                        pwt = psA.tile([128, 128], F32, tag="pwt")
                        nc.tensor.matmul(
                            pwt[:], wq_sb[io][:, ts(jo, 128)], identity[:],
                            start=True, stop=True,
                        )
                        nc.scalar.copy(wqT[jo][:, ts(io, 128)], pwt[:])

                query_sb = sa.tile([BPC, HID], F32, tag="query_sb")
                nc.sync.dma_start(out=query_sb[:], in_=q_d[:])
                qTq = [sa.tile([128, BPC], F32, name=f"qTq{j}", tag=f"qTq{j}") for j in range(4)]
                for jo in range(4):
                    pqt = psA.tile([128, BPC], F32, tag="pwt")
                    nc.tensor.matmul(
                        pqt[:], query_sb[:, ts(jo, 128)], identity[0:BPC, 0:BPC],
                        start=True, stop=True,
                    )
                    nc.scalar.copy(qTq[jo][:], pqt[:])
                # q = query @ Wq.T  ->  [4, 512]
                ps_q = psA.tile([BPC, HID], F32, tag="ps_q", bufs=1)
                for jo in range(4):
                    nc.tensor.matmul(
                        ps_q[:], qTq[jo][:], wqT[jo][:],
                        start=(jo == 0), stop=(jo == 3),
                    )
                nc.scalar.copy(q_sb[:], ps_q[:])
                # qts[d, h, b] = q[b, h*64+d]   (64 partitions)
                for h in range(NHEADS):
                    pqh = psA.tile([HEAD_DIM, BPC], F32, tag="pwt")
                    nc.tensor.matmul(
                        pqh[:], q_sb[:, ts(h, HEAD_DIM)], identity[0:BPC, 0:BPC],
                        start=True, stop=True,
                    )
                    nc.scalar.copy(qts[:, h, :], pqh[:])
                nc.scalar.copy(qts_bf[:], qts[:])

            # ---------------- setup phase B: qrep broadcast -----------------
            # qrep_bf[p, b, :] = q[b, :] on every partition: bounce q via
            # DRAM, then ONE sync-queue (HWDGE) broadcast DMA of the flat
            # [1, 2048] row to all 128 partitions + a single ACT cast. The
            # sync queue is free of the K/V flood, so this is ready early.
            qrep = persist.tile([128, BPC * HID], F32, tag="qrep")
            with tc.tile_pool(name="dramq", bufs=1, space="DRAM") as dq:
                q_dram = dq.tile([BPC, HID], F32, tag="q_dram")
                nc.sync.dma_start(out=q_dram[:], in_=q_sb[:])
                nc.sync.dma_start(
                    out=qrep[:],
                    in_=q_dram[:].rearrange("b f -> (b f)").partition_broadcast(128),
                )
            nc.scalar.copy(
                qrep_bf[:], qrep[:].rearrange("p (b f) -> p b f", b=BPC)
            )

            # positional scores for one head, software-pipelined one head
            # ahead of the row compute:
            # pos[p, mo, b] = sum_d key_pe[d, m] * q[b, h*64+d]
            # The stationary kpe slice is strided, so it is loaded as two
            # 64-column halves: a 128-column non-f32 stationary would engage
            # the compiler's fast-weight-load path, which assumes contiguous
            # weights.
            def emit_pos(h):
                mo_h = mohs[h]
                kpe_r = kpe_sb[:, mstarts[h]:M].rearrange(
                    "d (p mo) -> d mo p", mo=mo_h
                )
                ps_p = ps_pos_pool.tile([128, maxmo, BPC], F32, tag="ps_p")
                for mo in range(mo_h):
                    nc.tensor.matmul(
                        ps_p[0:64, mo, :], kpe_r[:, mo, 0:64], qts_bf[:, h, :],
                        start=True, stop=True,
                    )
                    nc.tensor.matmul(
                        ps_p[64:128, mo, :], kpe_r[:, mo, 64:128], qts_bf[:, h, :],
                        start=True, stop=True,
                    )
                nc.scalar.copy(pos_sb[h][:], ps_p[:, 0:mo_h, :])
                # fold the per-head log-mask into the positional scores
                nc.vector.tensor_add(
                    pos_sb[h][:], pos_sb[h][:],
                    lm8[h][:]
                    .rearrange("p (m x) -> p m x", x=1)
                    .broadcast_to((128, mo_h, BPC)),
                )

            emit_pos(order[0])

            # Wo transposes, deferred until after the first head's pos MMs
            # so nothing row-critical sits behind them on the PE stream
            # (woT is only consumed by the output projection at the end)
            with (
                tc.tile_pool(name="setupW", bufs=1) as sw,
                tc.tile_pool(name="psW", bufs=2, space="PSUM") as psW,
            ):
                wo_sb = [sw.tile([128, HID], F32, name=f"wo_sb{i}", tag="wo_sb", bufs=2) for i in range(4)]
                for i in range(4):
                    nc.sync.dma_start(out=wo_sb[i][:], in_=wo_d[ts(i, 128), :])
                for io in range(4):
                    for jo in range(4):
                        pwt2 = psW.tile([128, 128], F32, tag="pwt2")
                        nc.tensor.matmul(
                            pwt2[:], wo_sb[io][:, ts(jo, 128)], identity[:],
                            start=True, stop=True,
                        )
                        nc.scalar.copy(woT[jo][:, ts(io, 128)], pwt2[:])

            # ---------------- main loop: heads, then batches ----------------
            for j, h in enumerate(order):
                mo_h = mohs[h]
                mst = mstarts[h]

                if j + 2 < NHEADS:
                    emit_prefetch(order[j + 2])
                if j + 1 < NHEADS:
                    emit_pos(order[j + 1])

                for b in range(BPC):
                    kt, vt = kv_tiles[h][b]
                    if not USE_CAST_DMA:
                        vtb = kv_pool.tile([128, maxmo, HEAD_DIM], BF16, tag="vtb", bufs=2)
                        nc.scalar.copy(vtb[:, 0:mo_h, :], vt[:, 0:mo_h, :])
                        vt = vtb
                    # content scores: scores[p, mo] = sum_d key[..] * q[..]
                    prod = sc_pool.tile([128, maxmo, HEAD_DIM], BF16, tag="prod", bufs=2)
                    q_b = (
                        qrep_bf[:, b, ts(h, HEAD_DIM)]
                        .rearrange("p (x d) -> p x d", x=1)
                        .broadcast_to((128, mo_h, HEAD_DIM))
                    )
                    nc.vector.tensor_mul(prod[:, 0:mo_h, :], kt[:, 0:mo_h, :], q_b)
                    scores = sc_pool.tile([128, maxmo], F32, tag="scores")
                    nc.vector.reduce_sum(
                        scores[:, 0:mo_h], prod[:, 0:mo_h, :], axis=mybir.AxisListType.X
                    )
                    nc.vector.tensor_add(
                        scores[:, 0:mo_h], scores[:, 0:mo_h], pos_sb[h][:, :, b]
                    )
                    # w = exp((scores + 8 ln mask) / sqrt(d)) -- the mask
                    # multiply rides along inside the exp; Sigma_w comes out
                    # of the fused ACT accumulator. (The reference's extra
                    # 1e-8*Sigma_e denominator term is ~5e-6 relative and is
                    # dropped.)
                    sums = sc_pool.tile([128, 1], F32, tag="sums")
                    w_t = sc_pool.tile([128, maxmo], BF16, tag="w_t")
                    nc.scalar.activation(
                        out=w_t[:, 0:mo_h], in_=scores[:, 0:mo_h], func=AF.Exp,
                        scale=float(1.0 / math.sqrt(HEAD_DIM)),
                        accum_out=sums[:, 0:1],
                    )
                    # partition-reduce Sigma_w, then scal = 1/Sigma_w
                    ps_s = ps_s_pool.tile([1, 1], F32, tag="ps_s")
                    nc.tensor.matmul(
                        ps_s[:], ones_col[:], sums[:], start=True, stop=True
                    )
                    scal = sc_pool.tile([1, 1], F32, tag="scal")
                    nc.vector.reciprocal(scal[:], ps_s[:])
                    # out_row = sum_m w[m] * value[m, :]   (bf16 PE, PSUM accum)
                    ps_o = ps_o_pool.tile([1, HEAD_DIM], F32, tag="ps_o")
                    for mo in range(mo_h):
                        nc.tensor.matmul(
                            ps_o[:],
                            w_t[:, mo : mo + 1],
                            vt[:, mo, :],
                            start=(mo == 0),
                            stop=(mo == mo_h - 1),
                        )
                    # ao[0, b, h*64:(h+1)*64] = ps_o * scal
                    nc.scalar.activation(
                        out=ao_sb[0:1, b, ts(h, HEAD_DIM)], in_=ps_o[:],
                        func=AF.Copy, scale=scal[:, 0:1],
                    )

            # ---------------- output projection -------------------------
            with tc.tile_pool(name="ps_fin", bufs=1, space="PSUM") as ps_fin_pool:
                aoT = []
                for co in range(4):
                    ps_t2 = ps_fin_pool.tile([128, BPC], F32, name="ps_t2", tag="ps_fin")
                    for b in range(BPC):
                        nc.tensor.matmul(
                            ps_t2[:, b : b + 1],
                            ao_sb[0:1, b, ts(co, 128)],
                            identity[0:1, 0:1],
                            start=True, stop=True,
                        )
                    t_sb = fin_pool.tile([128, BPC], F32, name=f"t_sb{co}", tag=f"t_sb{co}")
                    nc.scalar.copy(t_sb[:], ps_t2[:])
                    aoT.append(t_sb)
                ps_f = ps_fin_pool.tile([BPC, HID], F32, name="ps_f", tag="ps_fin")
                for co in range(4):
                    nc.tensor.matmul(
                        ps_f[:], aoT[co][:], woT[co][:],
                        start=(co == 0), stop=(co == 3),
                    )
                out_sb = fin_pool.tile([BPC, HID], F32, tag="out_sb")
                nc.scalar.copy(out_sb[:], ps_f[:])
                nc.sync.dma_start(out=out_d[:], in_=out_sb[:])

    nc.compile()
    return nc


def _get_nc(mstarts):
    if mstarts not in _CACHE:
        _CACHE[mstarts] = build_nc(mstarts)
    return _CACHE[mstarts]


def _make_in_maps(query, key, value, Wq, Wo, key_pe, span):
    q2 = np.ascontiguousarray(np.asarray(query, np.float32).reshape(B, HID))
    key = np.asarray(key, np.float32)
    value = np.asarray(value, np.float32)
    Wq = np.ascontiguousarray(np.asarray(Wq, np.float32))
    Wo = np.ascontiguousarray(np.asarray(Wo, np.float32))
    key_pe = np.ascontiguousarray(np.asarray(key_pe, np.float32))
    span = np.ascontiguousarray(np.asarray(span, np.float32))
    in_maps = []
    for c in range(N_CORES):
        in_maps.append(
            {
                "query": np.ascontiguousarray(q2[c * BPC : (c + 1) * BPC]),
                "key": np.ascontiguousarray(key[c * NPC : (c + 1) * NPC]),
                "value": np.ascontiguousarray(value[c * NPC : (c + 1) * NPC]),
                "Wq": Wq,
                "Wo": Wo,
                "key_pe": key_pe,
                "span": span,
            }
        )
    return in_maps


def _install_ntff_hook():
    """Shim antenv.axon_hooks with a ctypes NTFF profile hook so
    run_bass_kernel_spmd(trace=True) works in this container."""
    import contextlib
    import ctypes
    import types

    try:
        import antenv.axon_hooks  # noqa: F401

        return
    except ImportError:
        pass
    so_path = "/opt/axon/libaxon_pjrt.so"
    import antenv

    mod = types.ModuleType("antenv.axon_hooks")
    holder = {"hook": None}

    if os.path.exists(so_path):
        lib = ctypes.CDLL(so_path)
        if hasattr(lib, "axon_start_nrt_profile"):
            lib.axon_start_nrt_profile.argtypes = [
                ctypes.POINTER(ctypes.c_int64),
                ctypes.c_size_t,
            ]
            lib.axon_start_nrt_profile.restype = ctypes.c_int64
            lib.axon_stop_nrt_profile.argtypes = [ctypes.c_char_p]
            lib.axon_stop_nrt_profile.restype = ctypes.c_int64

            @contextlib.contextmanager
            def _hook(output_dir, device_ids):
                import jax

                jax.devices()
                if device_ids:
                    ids = (ctypes.c_int64 * len(device_ids))(*device_ids)
                    rc = lib.axon_start_nrt_profile(ids, len(device_ids))
                else:
                    rc = lib.axon_start_nrt_profile(None, 0)
                if rc != 0:
                    raise RuntimeError(f"axon_start_nrt_profile rc={rc}")
                try:
                    yield
                finally:
                    n = lib.axon_stop_nrt_profile(str(output_dir).encode())
                    print(f"profile: {n} file(s) written to {output_dir}")

            holder["hook"] = _hook

    mod.get_axon_ntff_profile_hook = lambda: holder["hook"]
    mod.set_axon_ntff_profile_hook = lambda h: holder.__setitem__("hook", h)
    sys.modules["antenv.axon_hooks"] = mod
    antenv.axon_hooks = mod


def run(query, key, value, Wq, Wo, key_pe, span, trace=False):
    """Run on hardware; returns (output [B,1,HID], BassKernelResults)."""
    from concourse import bass_utils
    from concourse.bass_utils import run_bass_kernel_spmd

    if trace:
        _install_ntff_hook()
        bass_utils.upload_artifacts = lambda tmpdir: f"local:{tmpdir}"
    nc = _get_nc(compute_mstarts(span))
    in_maps = _make_in_maps(query, key, value, Wq, Wo, key_pe, span)
    res = run_bass_kernel_spmd(nc, in_maps, list(range(N_CORES)), trace=trace)
    out = np.concatenate(
        [np.asarray(res.results[c]["out"]) for c in range(N_CORES)], axis=0
    )
    return out.reshape(B, 1, HID).astype(np.float32), res


def kernel(query, key, value, Wq, Wo, key_pe, span):
    out, _ = run(query, key, value, Wq, Wo, key_pe, span, trace=False)
    return out


# revision 29
# speedup vs baseline: 1.0178x; 1.0178x over previous
"""Trainium2 Bass kernel for multi-head attention with adaptive span masking.

Computation (per the nn.Module):
    q = (query @ Wq.T) split into B*H rows of size d=64
    attn = softmax((key . q + q @ key_pe) / sqrt(d))
    attn = renormalize(attn * adaptive_span_mask)
    out = (attn . value) merged heads @ Wo.T

Key optimization: the adaptive-span mask is exactly zero for positions
m <= (M-1) - RAMP - span[h]*M, so those key/value rows contribute
nothing to the output (their only coupling is the 1e-8*sum(softmax)
term in the renormalization denominator, which perturbs the result by
~5e-6 relative). Each head therefore only reads the tail [mstart_h, M)
of key/value, cutting HBM traffic by ~2x. mstart_h is computed on the
host from the actual span input and baked into the compiled kernel.

Sharding: batch-parallel across 8 cores. Core c gets batches [4c, 4c+4)
(all 8 heads) = rows [32c, 32c+32) of key/value; Wq/Wo/key_pe/span are
replicated. Each core produces its own [4, 512] output block; the host
concatenates. No collectives needed.

Engine split per (batch, head) row:
  - key AND value loaded via gpsimd (SWDGE) DMA with inline f32->bf16
    cast: HBM reads stay f32 (unavoidable) but no engine time is spent
    casting and SBUF footprint halves
  - QK dot on DVE: bf16 multiply + reduce over d
  - positional scores precomputed per head on PE (key_pe stationary)
  - exp on ACT (with fused sum); mask-mult + sum fused in one DVE
    tensor_tensor_reduce; PV accumulation on PE in bf16
  - prefetch is software-pipelined two heads ahead so the DMA queues
    never drain; masks/iotas are computed in setup so the gpsimd queue
    carries only DMA work in the main loop
"""

import math
import os
import sys

import numpy as np

for _p in ("/opt/trn_rl_repo", "/root/.axon_site/_ro/trn_rl_repo"):
    if os.path.isdir(_p) and _p not in sys.path:
        sys.path.insert(0, _p)

import concourse.bass as bass
import concourse.bacc as bacc
import concourse.mybir as mybir
from concourse.bass import ts
from concourse.masks import make_identity
from concourse.tile import TileContext

F32 = mybir.dt.float32
BF16 = mybir.dt.bfloat16

# Problem constants (hardcoded per contest contract)
NHEADS = 8
HEAD_DIM = 64
HID = NHEADS * HEAD_DIM  # 512
B = 32
M = 8192
RAMP = 32.0

N_CORES = 8
BPC = B // N_CORES        # 4 batches per core
NPC = BPC * NHEADS        # 32 (b,h) rows per core

# tensor_tensor_reduce faults the runtime on this deployment; keep the
# two-op mul+reduce path (opt back in with K_TTR=1 to re-test)
USE_TTR = os.environ.get("K_TTR", "") != ""
USE_CAST_DMA = os.environ.get("K_NO_CAST", "") == ""

_CACHE = {}


def compute_mstarts(span: np.ndarray) -> tuple:
    """First key/value position with nonzero mask, per head, aligned
    down to a multiple of 128 (the SBUF partition count).

    mask[h, m] = clip((m - (M-1) + span[h]*M)/RAMP + 1, 0, 1) is zero
    iff m <= (M-1) - RAMP - span[h]*M.
    """
    s = np.asarray(span, np.float64).reshape(-1)
    last_zero = np.floor((M - 1) - RAMP - s * M).astype(np.int64)
    mstart = np.clip(last_zero, 0, M - 128)
    mstart = (mstart // 128) * 128
    return tuple(int(x) for x in mstart)


def build_nc(mstarts):
    nc = bacc.Bacc(None, target_bir_lowering=False)
    AF = mybir.ActivationFunctionType
    ALU = mybir.AluOpType

    mohs = [(M - mstarts[h]) // 128 for h in range(NHEADS)]
    maxmo = max(mohs)
    # big heads first (their DMA overlaps setup), smallest last (short tail)
    order = sorted(range(NHEADS), key=lambda h: -mohs[h])

    q_d = nc.dram_tensor("query", [BPC, HID], F32, kind="ExternalInput")
    k_d = nc.dram_tensor("key", [NPC, M, HEAD_DIM], F32, kind="ExternalInput")
    v_d = nc.dram_tensor("value", [NPC, M, HEAD_DIM], F32, kind="ExternalInput")
    wq_d = nc.dram_tensor("Wq", [HID, HID], F32, kind="ExternalInput")
    wo_d = nc.dram_tensor("Wo", [HID, HID], F32, kind="ExternalInput")
    kpe_d = nc.dram_tensor("key_pe", [HEAD_DIM, M], F32, kind="ExternalInput")
    span_d = nc.dram_tensor("span", [NHEADS, 1], F32, kind="ExternalInput")
    out_d = nc.dram_tensor("out", [BPC, HID], F32, kind="ExternalOutput")

    with TileContext(nc) as tc:
        with (
            tc.tile_pool(name="persist", bufs=1) as persist,
            # main-loop pools created BEFORE setup pools so the kv DMAs get
            # SBUF ranges disjoint from setup tiles (no WAR dep -> kv loads
            # start at t=0, overlapping the whole setup phase)
            tc.tile_pool(name="kv", bufs=8) as kv_pool,
            tc.tile_pool(name="sc", bufs=3) as sc_pool,
            tc.tile_pool(name="fin", bufs=1) as fin_pool,
            tc.tile_pool(name="ps_pos", bufs=2, space="PSUM") as ps_pos_pool,
            tc.tile_pool(name="ps_s", bufs=1, space="PSUM") as ps_s_pool,
            tc.tile_pool(name="ps_o", bufs=2, space="PSUM") as ps_o_pool,
        ):
            identity = persist.tile([128, 128], F32, tag="identity")
            make_identity(nc, identity[:])
            ones_row = persist.tile([1, 128], F32, tag="ones_row")
            nc.vector.memset(ones_row[:], 1.0)
            ones_col = persist.tile([128, 1], F32, tag="ones_col")
            nc.vector.memset(ones_col[:], 1.0)

            woT = [persist.tile([128, HID], F32, name=f"woT{j}", tag=f"woT{j}") for j in range(4)]
            q_sb = persist.tile([BPC, HID], F32, tag="q_sb")
            qts = persist.tile([HEAD_DIM, NHEADS, BPC], F32, tag="qts")
            qts_bf = persist.tile([HEAD_DIM, NHEADS, BPC], BF16, tag="qts_bf")
            qrep_bf = persist.tile([128, BPC, HID], BF16, tag="qrep_bf")
            kpe_sb = persist.tile([HEAD_DIM, M], BF16, tag="kpe_sb")
            span_b = persist.tile([128, NHEADS], F32, tag="span_b")
            span_row = persist.tile([1, NHEADS], F32, tag="span_row")
            pos_sb = [
                persist.tile([128, mohs[h], BPC], F32, name=f"pos{h}", tag=f"pos{h}")
                for h in range(NHEADS)
            ]
            masks = [
                persist.tile([128, mohs[h]], F32, name=f"mask{h}", tag=f"mask{h}")
                for h in range(NHEADS)
            ]
            lm8 = [
                persist.tile([128, mohs[h]], F32, name=f"lm8_{h}", tag=f"lm8_{h}")
                for h in range(NHEADS)
            ]
            ao_sb = persist.tile([1, BPC, HID], F32, tag="ao_sb")

            # key_pe tail (only columns any head can touch), cast to bf16
            # in-flight; first instruction on the gpsimd queue
            mstart_min = min(mstarts)
            nc.gpsimd.dma_start(
                out=kpe_sb[:, mstart_min:M], in_=kpe_d[:, mstart_min:M]
            )

            # ---- span bias + adaptive-span masks, computed FIRST so the
            # row pipeline is never gated on them (they feed every row's
            # mask multiply)
            nc.sync.dma_start(out=span_row[:], in_=span_d[:].rearrange("h o -> o h"))
            span_b2 = persist.tile([128, NHEADS], F32, tag="span_b2")
            with tc.tile_pool(name="ps_span", bufs=1, space="PSUM") as ps_span:
                ps_sp = ps_span.tile([128, NHEADS], F32, tag="ps_sp")
                nc.tensor.matmul(
                    ps_sp[:], ones_row[:], span_row[:], start=True, stop=True
                )
                # span_b[p, h] = span[h]*M/RAMP - (M-1)/RAMP + 1;
                # span_b2 folds in the per-head mstart/RAMP offset
                bias_const = float(-(M - 1) / RAMP + 1.0)
                nc.scalar.activation(
                    out=span_b[:], in_=ps_sp[:], func=AF.Copy,
                    scale=float(M / RAMP), bias=bias_const,
                )
            for h in range(NHEADS):
                nc.scalar.activation(
                    out=span_b2[:, h : h + 1], in_=span_b[:, h : h + 1],
                    func=AF.Copy, bias=float(mstarts[h] / RAMP),
                )
            # masks[h] = clip(..., 1e-30, 1); lm8[h] = 8*ln(masks[h]).
            # The mask multiply is folded into the exp: w = exp(s/8 + ln m)
            # = exp((s + 8 ln m) / 8), with 8*ln(m) pre-added into the
            # positional-score tile once per head. The 1e-30 floor keeps
            # ln() finite; exp then underflows those positions to exactly 0.
            for h in range(NHEADS):
                mo_h = mohs[h]
                m_f = sc_pool.tile([128, maxmo], F32, tag="m_f", bufs=4)
                nc.gpsimd.iota(
                    out=m_f[:, 0:mo_h], pattern=[[1, mo_h]], base=0,
                    channel_multiplier=mo_h,
                    allow_small_or_imprecise_dtypes=True,
                )
                nc.scalar.activation(
                    out=masks[h][:], in_=m_f[:, 0:mo_h], func=AF.Identity,
                    scale=float(1.0 / RAMP), bias=span_b2[:, h : h + 1],
                )
                nc.vector.tensor_scalar(
                    out=masks[h][:], in0=masks[h][:],
                    scalar1=1e-30, scalar2=1.0,
                    op0=ALU.max, op1=ALU.min,
                )
                nc.scalar.activation(
                    out=lm8[h][:], in_=masks[h][:], func=AF.Ln,
                )
                nc.vector.tensor_scalar(
                    out=lm8[h][:], in0=lm8[h][:],
                    scalar1=8.0, scalar2=0.0,
                    op0=ALU.mult, op1=ALU.add,
                )

            # K/V prefetch for one head (4 batch rows), f32->bf16 in-flight
            kv_tiles = {}

            def emit_prefetch(h):
                mo_h = mohs[h]
                mst = mstarts[h]
                tiles = []
                for b in range(BPC):
                    i = b * NHEADS + h
                    if USE_CAST_DMA:
                        kt = kv_pool.tile([128, maxmo, HEAD_DIM], BF16, tag="kt")
                        nc.gpsimd.dma_start(
                            out=kt[:, 0:mo_h, :],
                            in_=k_d[i, mst:M, :].rearrange("(p mo) d -> p mo d", p=128),
                        )
                        vt = kv_pool.tile([128, maxmo, HEAD_DIM], BF16, tag="vt")
                        nc.gpsimd.dma_start(
                            out=vt[:, 0:mo_h, :],
                            in_=v_d[i, mst:M, :].rearrange("(p mo) d -> p mo d", p=128),
                        )
                    else:
                        kt = kv_pool.tile([128, maxmo, HEAD_DIM], F32, tag="kt", bufs=4)
                        nc.sync.dma_start(
                            out=kt[:, 0:mo_h, :],
                            in_=k_d[i, mst:M, :].rearrange("(p mo) d -> p mo d", p=128),
                        )
                        vt = kv_pool.tile([128, maxmo, HEAD_DIM], F32, tag="vt", bufs=4)
                        nc.scalar.dma_start(
                            out=vt[:, 0:mo_h, :],
                            in_=v_d[i, mst:M, :].rearrange("(p mo) d -> p mo d", p=128),
                        )
                    tiles.append((kt, vt))
                kv_tiles[h] = tiles

            emit_prefetch(order[0])
            emit_prefetch(order[1])

            # ---------------- setup phase A: weight transposes + q ----------
            with (
                tc.tile_pool(name="setupA", bufs=1) as sa,
                tc.tile_pool(name="psA", bufs=2, space="PSUM") as psA,
            ):
                wqT = [sa.tile([128, HID], F32, name=f"wqT{j}", tag=f"wqT{j}") for j in range(4)]
                wq_sb = [sa.tile([128, HID], F32, name=f"wq_sb{i}", tag="wq_sb", bufs=2) for i in range(4)]
                for i in range(4):
                    nc.sync.dma_start(out=wq_sb[i][:], in_=wq_d[ts(i, 128), :])
                for io in range(4):
                    for jo in range(4):
                        pwt = psA.tile([128, 128], F32, tag="pwt")
                        nc.tensor.matmul(
                            pwt[:], wq_sb[io][:, ts(jo, 128)], identity[:],
                            start=True, stop=True,
                        )
                        nc.scalar.copy(wqT[jo][:, ts(io, 128)], pwt[:])

                query_sb = sa.tile([BPC, HID], F32, tag="query_sb")
                nc.sync.dma_start(out=query_sb[:], in_=q_d[:])
                qTq = [sa.tile([128, BPC], F32, name=f"qTq{j}", tag=f"qTq{j}") for j in range(4)]
                for jo in range(4):
                    pqt = psA.tile([128, BPC], F32, tag="pwt")
                    nc.tensor.matmul(
                        pqt[:], query_sb[:, ts(jo, 128)], identity[0:BPC, 0:BPC],
                        start=True, stop=True,
                    )
                    nc.scalar.copy(qTq[jo][:], pqt[:])
                # q = query @ Wq.T  ->  [4, 512]
                ps_q = psA.tile([BPC, HID], F32, tag="ps_q", bufs=1)
                for jo in range(4):
                    nc.tensor.matmul(
                        ps_q[:], qTq[jo][:], wqT[jo][:],
                        start=(jo == 0), stop=(jo == 3),
                    )
                nc.scalar.copy(q_sb[:], ps_q[:])
                # qts[d, h, b] = q[b, h*64+d]   (64 partitions)
                for h in range(NHEADS):
                    pqh = psA.tile([HEAD_DIM, BPC], F32, tag="pwt")
                    nc.tensor.matmul(
                        pqh[:], q_sb[:, ts(h, HEAD_DIM)], identity[0:BPC, 0:BPC],
                        start=True, stop=True,
                    )
                    nc.scalar.copy(qts[:, h, :], pqh[:])
                nc.scalar.copy(qts_bf[:], qts[:])

            # ---------------- setup phase B: qrep broadcast -----------------
            # qrep_bf[p, b, :] = q[b, :] on every partition: bounce q via
            # DRAM, then ONE sync-queue (HWDGE) broadcast DMA of the flat
            # [1, 2048] row to all 128 partitions + a single ACT cast. The
            # sync queue is free of the K/V flood, so this is ready early.
            qrep = persist.tile([128, BPC * HID], F32, tag="qrep")
            with tc.tile_pool(name="dramq", bufs=1, space="DRAM") as dq:
                q_dram = dq.tile([BPC, HID], F32, tag="q_dram")
                nc.sync.dma_start(out=q_dram[:], in_=q_sb[:])
                nc.sync.dma_start(
                    out=qrep[:],
                    in_=q_dram[:].rearrange("b f -> (b f)").partition_broadcast(128),
                )
            # cast on the DVE: the scalar engine's stream fills up with
            # PSUM->SBUF pos copies and the list scheduler pushed this cast
            # behind all of them, gating every row multiply until ~96us
            nc.vector.tensor_scalar(
                out=qrep_bf[:], in0=qrep[:].rearrange("p (b f) -> p b f", b=BPC),
                scalar1=1.0, scalar2=0.0, op0=ALU.mult, op1=ALU.add,
            )

            # positional scores for one head, software-pipelined one head
            # ahead of the row compute:
            # pos[p, mo, b] = sum_d key_pe[d, m] * q[b, h*64+d]
            # The stationary kpe slice is strided, so it is loaded as two
            # 64-column halves: a 128-column non-f32 stationary would engage
            # the compiler's fast-weight-load path, which assumes contiguous
            # weights.
            def emit_pos(h):
                mo_h = mohs[h]
                kpe_r = kpe_sb[:, mstarts[h]:M].rearrange(
                    "d (p mo) -> d mo p", mo=mo_h
                )
                ps_p = ps_pos_pool.tile([128, maxmo, BPC], F32, tag="ps_p")
                for mo in range(mo_h):
                    nc.tensor.matmul(
                        ps_p[0:64, mo, :], kpe_r[:, mo, 0:64], qts_bf[:, h, :],
                        start=True, stop=True,
                    )
                    nc.tensor.matmul(
                        ps_p[64:128, mo, :], kpe_r[:, mo, 64:128], qts_bf[:, h, :],
                        start=True, stop=True,
                    )
                nc.scalar.copy(pos_sb[h][:], ps_p[:, 0:mo_h, :])
                # fold the per-head log-mask into the positional scores
                nc.vector.tensor_add(
                    pos_sb[h][:], pos_sb[h][:],
                    lm8[h][:]
                    .rearrange("p (m x) -> p m x", x=1)
                    .broadcast_to((128, mo_h, BPC)),
                )

            emit_pos(order[0])

            # Wo transposes, deferred until after the first head's pos MMs
            # so nothing row-critical sits behind them on the PE stream
            # (woT is only consumed by the output projection at the end)
            with (
                tc.tile_pool(name="setupW", bufs=1) as sw,
                tc.tile_pool(name="psW", bufs=2, space="PSUM") as psW,
            ):
                wo_sb = [sw.tile([128, HID], F32, name=f"wo_sb{i}", tag="wo_sb", bufs=2) for i in range(4)]
                for i in range(4):
                    nc.sync.dma_start(out=wo_sb[i][:], in_=wo_d[ts(i, 128), :])
                for io in range(4):
                    for jo in range(4):
                        pwt2 = psW.tile([128, 128], F32, tag="pwt2")
                        nc.tensor.matmul(
                            pwt2[:], wo_sb[io][:, ts(jo, 128)], identity[:],
                            start=True, stop=True,
                        )
                        nc.scalar.copy(woT[jo][:, ts(io, 128)], pwt2[:])

            # ---------------- main loop: heads, then batches ----------------
            for j, h in enumerate(order):
                mo_h = mohs[h]
                mst = mstarts[h]

                if j + 2 < NHEADS:
                    emit_prefetch(order[j + 2])
                if j + 1 < NHEADS:
                    emit_pos(order[j + 1])

                for b in range(BPC):
                    kt, vt = kv_tiles[h][b]
                    if not USE_CAST_DMA:
                        vtb = kv_pool.tile([128, maxmo, HEAD_DIM], BF16, tag="vtb", bufs=2)
                        nc.scalar.copy(vtb[:, 0:mo_h, :], vt[:, 0:mo_h, :])
                        vt = vtb
                    # content scores: scores[p, mo] = sum_d key[..] * q[..]
                    prod = sc_pool.tile([128, maxmo, HEAD_DIM], BF16, tag="prod", bufs=2)
                    q_b = (
                        qrep_bf[:, b, ts(h, HEAD_DIM)]
                        .rearrange("p (x d) -> p x d", x=1)
                        .broadcast_to((128, mo_h, HEAD_DIM))
                    )
                    nc.vector.tensor_mul(prod[:, 0:mo_h, :], kt[:, 0:mo_h, :], q_b)
                    scores = sc_pool.tile([128, maxmo], F32, tag="scores")
                    nc.vector.reduce_sum(
                        scores[:, 0:mo_h], prod[:, 0:mo_h, :], axis=mybir.AxisListType.X
                    )
                    nc.vector.tensor_add(
                        scores[:, 0:mo_h], scores[:, 0:mo_h], pos_sb[h][:, :, b]
                    )
                    # w = exp((scores + 8 ln mask) / sqrt(d)) -- the mask
                    # multiply rides along inside the exp; Sigma_w comes out
                    # of the fused ACT accumulator. (The reference's extra
                    # 1e-8*Sigma_e denominator term is ~5e-6 relative and is
                    # dropped.)
                    sums = sc_pool.tile([128, 1], F32, tag="sums")
                    w_t = sc_pool.tile([128, maxmo], BF16, tag="w_t")
                    nc.scalar.activation(
                        out=w_t[:, 0:mo_h], in_=scores[:, 0:mo_h], func=AF.Exp,
                        scale=float(1.0 / math.sqrt(HEAD_DIM)),
                        accum_out=sums[:, 0:1],
                    )
                    # partition-reduce Sigma_w, then scal = 1/Sigma_w
                    ps_s = ps_s_pool.tile([1, 1], F32, tag="ps_s")
                    nc.tensor.matmul(
                        ps_s[:], ones_col[:], sums[:], start=True, stop=True
                    )
                    scal = sc_pool.tile([1, 1], F32, tag="scal")
                    nc.vector.reciprocal(scal[:], ps_s[:])
                    # out_row = sum_m w[m] * value[m, :]   (bf16 PE, PSUM accum)
                    ps_o = ps_o_pool.tile([1, HEAD_DIM], F32, tag="ps_o")
                    for mo in range(mo_h):
                        nc.tensor.matmul(
                            ps_o[:],
                            w_t[:, mo : mo + 1],
                            vt[:, mo, :],
                            start=(mo == 0),
                            stop=(mo == mo_h - 1),
                        )
                    # ao[0, b, h*64:(h+1)*64] = ps_o * scal
                    nc.scalar.activation(
                        out=ao_sb[0:1, b, ts(h, HEAD_DIM)], in_=ps_o[:],
                        func=AF.Copy, scale=scal[:, 0:1],
                    )

            # ---------------- output projection -------------------------
            with tc.tile_pool(name="ps_fin", bufs=1, space="PSUM") as ps_fin_pool:
                aoT = []
                for co in range(4):
                    ps_t2 = ps_fin_pool.tile([128, BPC], F32, name="ps_t2", tag="ps_fin")
                    for b in range(BPC):
                        nc.tensor.matmul(
                            ps_t2[:, b : b + 1],
                            ao_sb[0:1, b, ts(co, 128)],
                            identity[0:1, 0:1],
                            start=True, stop=True,
                        )
                    t_sb = fin_pool.tile([128, BPC], F32, name=f"t_sb{co}", tag=f"t_sb{co}")
                    nc.scalar.copy(t_sb[:], ps_t2[:])
                    aoT.append(t_sb)
                ps_f = ps_fin_pool.tile([BPC, HID], F32, name="ps_f", tag="ps_fin")
                for co in range(4):
                    nc.tensor.matmul(
                        ps_f[:], aoT[co][:], woT[co][:],
                        start=(co == 0), stop=(co == 3),
                    )
                out_sb = fin_pool.tile([BPC, HID], F32, tag="out_sb")
                nc.scalar.copy(out_sb[:], ps_f[:])
                nc.sync.dma_start(out=out_d[:], in_=out_sb[:])

    nc.compile()
    return nc


def _get_nc(mstarts):
    if mstarts not in _CACHE:
        _CACHE[mstarts] = build_nc(mstarts)
    return _CACHE[mstarts]


def _make_in_maps(query, key, value, Wq, Wo, key_pe, span):
    q2 = np.ascontiguousarray(np.asarray(query, np.float32).reshape(B, HID))
    key = np.asarray(key, np.float32)
    value = np.asarray(value, np.float32)
    Wq = np.ascontiguousarray(np.asarray(Wq, np.float32))
    Wo = np.ascontiguousarray(np.asarray(Wo, np.float32))
    key_pe = np.ascontiguousarray(np.asarray(key_pe, np.float32))
    span = np.ascontiguousarray(np.asarray(span, np.float32))
    in_maps = []
    for c in range(N_CORES):
        in_maps.append(
            {
                "query": np.ascontiguousarray(q2[c * BPC : (c + 1) * BPC]),
                "key": np.ascontiguousarray(key[c * NPC : (c + 1) * NPC]),
                "value": np.ascontiguousarray(value[c * NPC : (c + 1) * NPC]),
                "Wq": Wq,
                "Wo": Wo,
                "key_pe": key_pe,
                "span": span,
            }
        )
    return in_maps


def _install_ntff_hook():
    """Shim antenv.axon_hooks with a ctypes NTFF profile hook so
    run_bass_kernel_spmd(trace=True) works in this container."""
    import contextlib
    import ctypes
    import types

    try:
        import antenv.axon_hooks  # noqa: F401

        return
    except ImportError:
        pass
    so_path = "/opt/axon/libaxon_pjrt.so"
    import antenv

    mod = types.ModuleType("antenv.axon_hooks")
    holder = {"hook": None}

    if os.path.exists(so_path):
        lib = ctypes.CDLL(so_path)
        if hasattr(lib, "axon_start_nrt_profile"):
            lib.axon_start_nrt_profile.argtypes = [
                ctypes.POINTER(ctypes.c_int64),
                ctypes.c_size_t,
            ]
            lib.axon_start_nrt_profile.restype = ctypes.c_int64
            lib.axon_stop_nrt_profile.argtypes = [ctypes.c_char_p]
            lib.axon_stop_nrt_profile.restype = ctypes.c_int64

            @contextlib.contextmanager
            def _hook(output_dir, device_ids):
                import jax

                jax.devices()
                if device_ids:
                    ids = (ctypes.c_int64 * len(device_ids))(*device_ids)
                    rc = lib.axon_start_nrt_profile(ids, len(device_ids))
                else:
                    rc = lib.axon_start_nrt_profile(None, 0)
                if rc != 0:
                    raise RuntimeError(f"axon_start_nrt_profile rc={rc}")
                try:
                    yield
                finally:
                    n = lib.axon_stop_nrt_profile(str(output_dir).encode())
                    print(f"profile: {n} file(s) written to {output_dir}")

            holder["hook"] = _hook

    mod.get_axon_ntff_profile_hook = lambda: holder["hook"]
    mod.set_axon_ntff_profile_hook = lambda h: holder.__setitem__("hook", h)
    sys.modules["antenv.axon_hooks"] = mod
    antenv.axon_hooks = mod


def run(query, key, value, Wq, Wo, key_pe, span, trace=False):
    """Run on hardware; returns (output [B,1,HID], BassKernelResults)."""
    from concourse import bass_utils
    from concourse.bass_utils import run_bass_kernel_spmd

    if trace:
        _install_ntff_hook()
        bass_utils.upload_artifacts = lambda tmpdir: f"local:{tmpdir}"
    nc = _get_nc(compute_mstarts(span))
    in_maps = _make_in_maps(query, key, value, Wq, Wo, key_pe, span)
    res = run_bass_kernel_spmd(nc, in_maps, list(range(N_CORES)), trace=trace)
    out = np.concatenate(
        [np.asarray(res.results[c]["out"]) for c in range(N_CORES)], axis=0
    )
    return out.reshape(B, 1, HID).astype(np.float32), res


def kernel(query, key, value, Wq, Wo, key_pe, span):
    out, _ = run(query, key, value, Wq, Wo, key_pe, span, trace=False)
    return out


# revision 38
# speedup vs baseline: 1.0889x; 1.0698x over previous
"""Trainium2 Bass kernel for multi-head attention with adaptive span masking.

Computation (per the nn.Module):
    q = (query @ Wq.T) split into B*H rows of size d=64
    attn = softmax((key . q + q @ key_pe) / sqrt(d))
    attn = renormalize(attn * adaptive_span_mask)
    out = (attn . value) merged heads @ Wo.T

Key optimization: the adaptive-span mask is exactly zero for positions
m <= (M-1) - RAMP - span[h]*M, so those key/value rows contribute
nothing to the output (their only coupling is the 1e-8*sum(softmax)
term in the renormalization denominator, which perturbs the result by
~5e-6 relative). Each head therefore only reads the tail [mstart_h, M)
of key/value, cutting HBM traffic by ~2x. mstart_h is computed on the
host from the actual span input and baked into the compiled kernel.

Sharding: batch-parallel across 8 cores. Core c gets batches [4c, 4c+4)
(all 8 heads) = rows [32c, 32c+32) of key/value; Wq/Wo/key_pe/span are
replicated. Each core produces its own [4, 512] output block; the host
concatenates. No collectives needed.

Engine split per (batch, head) row:
  - key AND value loaded via gpsimd (SWDGE) DMA with inline f32->bf16
    cast: HBM reads stay f32 (unavoidable) but no engine time is spent
    casting and SBUF footprint halves
  - QK dot on DVE: bf16 multiply + reduce over d
  - positional scores precomputed per head on PE (key_pe stationary)
  - exp on ACT (with fused sum); mask-mult + sum fused in one DVE
    tensor_tensor_reduce; PV accumulation on PE in bf16
  - prefetch is software-pipelined two heads ahead so the DMA queues
    never drain; masks/iotas are computed in setup so the gpsimd queue
    carries only DMA work in the main loop
"""

import math
import os
import sys

import numpy as np

for _p in ("/opt/trn_rl_repo", "/root/.axon_site/_ro/trn_rl_repo"):
    if os.path.isdir(_p) and _p not in sys.path:
        sys.path.insert(0, _p)

import concourse.bass as bass
import concourse.bacc as bacc
import concourse.mybir as mybir
from concourse.bass import ts
from concourse.masks import make_identity
from concourse.tile import TileContext

F32 = mybir.dt.float32
BF16 = mybir.dt.bfloat16

# Problem constants (hardcoded per contest contract)
NHEADS = 8
HEAD_DIM = 64
HID = NHEADS * HEAD_DIM  # 512
B = 32
M = 8192
RAMP = 32.0

N_CORES = 8
BPC = B // N_CORES        # 4 batches per core
NPC = BPC * NHEADS        # 32 (b,h) rows per core

# tensor_tensor_reduce faults the runtime on this deployment; keep the
# two-op mul+reduce path (opt back in with K_TTR=1 to re-test)
USE_TTR = os.environ.get("K_TTR", "") != ""
USE_CAST_DMA = os.environ.get("K_NO_CAST", "") == ""

_CACHE = {}


def compute_mstarts(span: np.ndarray) -> tuple:
    """First key/value position with nonzero mask, per head, aligned
    down to a multiple of 128 (the SBUF partition count).

    mask[h, m] = clip((m - (M-1) + span[h]*M)/RAMP + 1, 0, 1) is zero
    iff m <= (M-1) - RAMP - span[h]*M.
    """
    s = np.asarray(span, np.float64).reshape(-1)
    last_zero = np.floor((M - 1) - RAMP - s * M).astype(np.int64)
    mstart = np.clip(last_zero, 0, M - 128)
    mstart = (mstart // 128) * 128
    return tuple(int(x) for x in mstart)


def build_nc(mstarts):
    nc = bacc.Bacc(None, target_bir_lowering=False)
    AF = mybir.ActivationFunctionType
    ALU = mybir.AluOpType

    mohs = [(M - mstarts[h]) // 128 for h in range(NHEADS)]
    maxmo = max(mohs)
    # big heads first (their DMA overlaps setup), smallest last (short tail)
    order = sorted(range(NHEADS), key=lambda h: -mohs[h])

    q_d = nc.dram_tensor("query", [BPC, HID], F32, kind="ExternalInput")
    k_d = nc.dram_tensor("key", [NPC, M, HEAD_DIM], F32, kind="ExternalInput")
    v_d = nc.dram_tensor("value", [NPC, M, HEAD_DIM], F32, kind="ExternalInput")
    wq_d = nc.dram_tensor("Wq", [HID, HID], F32, kind="ExternalInput")
    wo_d = nc.dram_tensor("Wo", [HID, HID], F32, kind="ExternalInput")
    kpe_d = nc.dram_tensor("key_pe", [HEAD_DIM, M], F32, kind="ExternalInput")
    span_d = nc.dram_tensor("span", [NHEADS, 1], F32, kind="ExternalInput")
    out_d = nc.dram_tensor("out", [BPC, HID], F32, kind="ExternalOutput")

    with TileContext(nc) as tc:
        with (
            tc.tile_pool(name="persist", bufs=1) as persist,
            # main-loop pools created BEFORE setup pools so the kv DMAs get
            # SBUF ranges disjoint from setup tiles (no WAR dep -> kv loads
            # start at t=0, overlapping the whole setup phase)
            tc.tile_pool(name="kv", bufs=8) as kv_pool,
            tc.tile_pool(name="sc", bufs=3) as sc_pool,
            tc.tile_pool(name="fin", bufs=1) as fin_pool,
            tc.tile_pool(name="ps_pos", bufs=2, space="PSUM") as ps_pos_pool,
            tc.tile_pool(name="ps_s", bufs=1, space="PSUM") as ps_s_pool,
            tc.tile_pool(name="ps_o", bufs=2, space="PSUM") as ps_o_pool,
        ):
            identity = persist.tile([128, 128], F32, tag="identity")
            make_identity(nc, identity[:])
            identity_bf = persist.tile([128, 128], BF16, tag="identity_bf")
            nc.vector.tensor_scalar(
                out=identity_bf[:], in0=identity[:],
                scalar1=1.0, scalar2=0.0, op0=ALU.mult, op1=ALU.add,
            )
            ones_row = persist.tile([1, 128], F32, tag="ones_row")
            nc.vector.memset(ones_row[:], 1.0)
            ones_col = persist.tile([128, 1], F32, tag="ones_col")
            nc.vector.memset(ones_col[:], 1.0)

            woT = [persist.tile([128, HID], BF16, name=f"woT{j}", tag=f"woT{j}") for j in range(4)]
            q_sb = persist.tile([BPC, HID], F32, tag="q_sb")
            qts = persist.tile([HEAD_DIM, NHEADS, BPC], F32, tag="qts")
            qts_bf = persist.tile([HEAD_DIM, NHEADS, BPC], BF16, tag="qts_bf")
            qrep_bf = persist.tile([128, BPC, HID], BF16, tag="qrep_bf")
            kpe_sb = persist.tile([HEAD_DIM, M], BF16, tag="kpe_sb")
            span_b = persist.tile([128, NHEADS], F32, tag="span_b")
            span_row = persist.tile([1, NHEADS], F32, tag="span_row")
            pos_sb = [
                persist.tile([128, mohs[h], BPC], F32, name=f"pos{h}", tag=f"pos{h}")
                for h in range(NHEADS)
            ]
            masks = [
                persist.tile([128, mohs[h]], F32, name=f"mask{h}", tag=f"mask{h}")
                for h in range(NHEADS)
            ]
            ao_sb = persist.tile([1, BPC, HID], F32, tag="ao_sb")

            # key_pe tail (only columns any head can touch), cast to bf16
            # in-flight; first instruction on the gpsimd queue. The weight
            # matrices and query also come in as bf16 cast-DMAs: their
            # transposes then run as cheap single-pass bf16 matmuls instead
            # of multi-pass fp32 ones, collapsing the PE front-end that every
            # downstream chain waits on.
            mstart_min = min(mstarts)
            nc.gpsimd.dma_start(
                out=kpe_sb[:, mstart_min:M], in_=kpe_d[:, mstart_min:M]
            )
            wq_bf = [persist.tile([128, HID], BF16, name=f"wq_bf{i}", tag=f"wq_bf{i}") for i in range(4)]
            wo_bf = [persist.tile([128, HID], BF16, name=f"wo_bf{i}", tag=f"wo_bf{i}") for i in range(4)]
            query_bf = persist.tile([BPC, HID], BF16, tag="query_bf")
            for i in range(4):
                nc.gpsimd.dma_start(out=wq_bf[i][:], in_=wq_d[ts(i, 128), :])
            nc.gpsimd.dma_start(out=query_bf[:], in_=q_d[:])
            for i in range(4):
                nc.gpsimd.dma_start(out=wo_bf[i][:], in_=wo_d[ts(i, 128), :])

            # ---- span bias + adaptive-span masks, computed FIRST so the
            # row pipeline is never gated on them (they feed every row's
            # mask multiply)
            nc.sync.dma_start(out=span_row[:], in_=span_d[:].rearrange("h o -> o h"))
            span_b2 = persist.tile([128, NHEADS], F32, tag="span_b2")
            with tc.tile_pool(name="ps_span", bufs=1, space="PSUM") as ps_span:
                ps_sp = ps_span.tile([128, NHEADS], F32, tag="ps_sp")
                nc.tensor.matmul(
                    ps_sp[:], ones_row[:], span_row[:], start=True, stop=True
                )
                # span_b[p, h] = span[h]*M/RAMP - (M-1)/RAMP + 1;
                # span_b2 folds in the per-head mstart/RAMP offset
                bias_const = float(-(M - 1) / RAMP + 1.0)
                nc.scalar.activation(
                    out=span_b[:], in_=ps_sp[:], func=AF.Copy,
                    scale=float(M / RAMP), bias=bias_const,
                )
            for h in range(NHEADS):
                nc.scalar.activation(
                    out=span_b2[:, h : h + 1], in_=span_b[:, h : h + 1],
                    func=AF.Copy, bias=float(mstarts[h] / RAMP),
                )
            # masks[h][p, j] = clip((mstart_h + p*mo_h + j)/RAMP
            #                       + span_b[h], 0, 1)
            # (applied per row as a cheap [128, mo_h] multiply whose only
            # dependency is this early block -- keeping the vector stream's
            # early ops shallow so the scheduler cannot stall it)
            for h in range(NHEADS):
                mo_h = mohs[h]
                m_f = sc_pool.tile([128, maxmo], F32, tag="m_f", bufs=4)
                nc.gpsimd.iota(
                    out=m_f[:, 0:mo_h], pattern=[[1, mo_h]], base=0,
                    channel_multiplier=mo_h,
                    allow_small_or_imprecise_dtypes=True,
                )
                nc.scalar.activation(
                    out=masks[h][:], in_=m_f[:, 0:mo_h], func=AF.Identity,
                    scale=float(1.0 / RAMP), bias=span_b2[:, h : h + 1],
                )
                nc.vector.tensor_scalar(
                    out=masks[h][:], in0=masks[h][:],
                    scalar1=0.0, scalar2=1.0,
                    op0=ALU.max, op1=ALU.min,
                )

            # K/V prefetch for one head (4 batch rows), f32->bf16 in-flight
            kv_tiles = {}

            def emit_prefetch(h):
                mo_h = mohs[h]
                mst = mstarts[h]
                tiles = []
                for b in range(BPC):
                    i = b * NHEADS + h
                    if USE_CAST_DMA:
                        kt = kv_pool.tile([128, maxmo, HEAD_DIM], BF16, tag="kt")
                        nc.gpsimd.dma_start(
                            out=kt[:, 0:mo_h, :],
                            in_=k_d[i, mst:M, :].rearrange("(p mo) d -> p mo d", p=128),
                        )
                        vt = kv_pool.tile([128, maxmo, HEAD_DIM], BF16, tag="vt")
                        nc.gpsimd.dma_start(
                            out=vt[:, 0:mo_h, :],
                            in_=v_d[i, mst:M, :].rearrange("(p mo) d -> p mo d", p=128),
                        )
                    else:
                        kt = kv_pool.tile([128, maxmo, HEAD_DIM], F32, tag="kt", bufs=4)
                        nc.sync.dma_start(
                            out=kt[:, 0:mo_h, :],
                            in_=k_d[i, mst:M, :].rearrange("(p mo) d -> p mo d", p=128),
                        )
                        vt = kv_pool.tile([128, maxmo, HEAD_DIM], F32, tag="vt", bufs=4)
                        nc.scalar.dma_start(
                            out=vt[:, 0:mo_h, :],
                            in_=v_d[i, mst:M, :].rearrange("(p mo) d -> p mo d", p=128),
                        )
                    tiles.append((kt, vt))
                kv_tiles[h] = tiles

            emit_prefetch(order[0])
            emit_prefetch(order[1])

            # ---------------- setup phase A: weight transposes + q ----------
            with (
                tc.tile_pool(name="setupA", bufs=1) as sa,
                tc.tile_pool(name="psA", bufs=2, space="PSUM") as psA,
            ):
                # all transposes as single-pass bf16 matmuls with 64-column
                # stationary halves (no fp32 multi-pass, no 128-col FWL)
                def transpose_blk(dst_slice, src_blk, pool):
                    # src_blk [128, 128] bf16 -> dst [128, 128] = src^T
                    pwt = pool.tile([128, 128], F32, tag="pwt")
                    nc.tensor.matmul(
                        pwt[0:64, :], src_blk[:, 0:64], identity_bf[:],
                        start=True, stop=True,
                    )
                    nc.tensor.matmul(
                        pwt[64:128, :], src_blk[:, 64:128], identity_bf[:],
                        start=True, stop=True,
                    )
                    nc.scalar.copy(dst_slice, pwt[:])

                wqT = [sa.tile([128, HID], BF16, name=f"wqT{j}", tag=f"wqT{j}") for j in range(4)]
                for io in range(4):
                    for jo in range(4):
                        transpose_blk(
                            wqT[jo][:, ts(io, 128)],
                            wq_bf[io][:, ts(jo, 128)], psA,
                        )
                qTq = [sa.tile([128, BPC], BF16, name=f"qTq{j}", tag=f"qTq{j}") for j in range(4)]
                for jo in range(4):
                    pqt = psA.tile([128, BPC], F32, tag="pwt")
                    nc.tensor.matmul(
                        pqt[0:64, :], query_bf[:, ts(2 * jo, 64)],
                        identity_bf[0:BPC, 0:BPC],
                        start=True, stop=True,
                    )
                    nc.tensor.matmul(
                        pqt[64:128, :], query_bf[:, ts(2 * jo + 1, 64)],
                        identity_bf[0:BPC, 0:BPC],
                        start=True, stop=True,
                    )
                    nc.scalar.copy(qTq[jo][:], pqt[:])
                # q = query @ Wq.T  ->  [4, 512]
                ps_q = psA.tile([BPC, HID], F32, tag="ps_q", bufs=1)
                for jo in range(4):
                    nc.tensor.matmul(
                        ps_q[:], qTq[jo][:], wqT[jo][:],
                        start=(jo == 0), stop=(jo == 3),
                    )
                nc.scalar.copy(q_sb[:], ps_q[:])
                # qts[d, h, b] = q[b, h*64+d]   (64 partitions)
                for h in range(NHEADS):
                    pqh = psA.tile([HEAD_DIM, BPC], F32, tag="pwt")
                    nc.tensor.matmul(
                        pqh[:], q_sb[:, ts(h, HEAD_DIM)], identity[0:BPC, 0:BPC],
                        start=True, stop=True,
                    )
                    nc.scalar.copy(qts[:, h, :], pqh[:])
                nc.scalar.copy(qts_bf[:], qts[:])

                # Wo transposes (bf16, only consumed by the final projection)
                for io in range(4):
                    for jo in range(4):
                        transpose_blk(
                            woT[jo][:, ts(io, 128)],
                            wo_bf[io][:, ts(jo, 128)], psA,
                        )

            # ---------------- setup phase B: qrep broadcast -----------------
            # qrep_bf[p, b, :] = q[b, :] on every partition: bounce q via
            # DRAM, then ONE sync-queue (HWDGE) broadcast DMA of the flat
            # [1, 2048] row to all 128 partitions + a single ACT cast. The
            # sync queue is free of the K/V flood, so this is ready early.
            qrep = persist.tile([128, BPC * HID], F32, tag="qrep")
            with tc.tile_pool(name="dramq", bufs=1, space="DRAM") as dq:
                q_dram = dq.tile([BPC, HID], F32, tag="q_dram")
                nc.sync.dma_start(out=q_dram[:], in_=q_sb[:])
                nc.sync.dma_start(
                    out=qrep[:],
                    in_=q_dram[:].rearrange("b f -> (b f)").partition_broadcast(128),
                )
            # cast on the DVE: the scalar engine's stream fills up with
            # PSUM->SBUF pos copies and the list scheduler pushed this cast
            # behind all of them, gating every row multiply until ~96us
            nc.vector.tensor_scalar(
                out=qrep_bf[:], in0=qrep[:].rearrange("p (b f) -> p b f", b=BPC),
                scalar1=1.0, scalar2=0.0, op0=ALU.mult, op1=ALU.add,
            )

            # positional scores for one head, software-pipelined one head
            # ahead of the row compute:
            # pos[p, mo, b] = sum_d key_pe[d, m] * q[b, h*64+d]
            # The stationary kpe slice is strided, so it is loaded as two
            # 64-column halves: a 128-column non-f32 stationary would engage
            # the compiler's fast-weight-load path, which assumes contiguous
            # weights.
            def emit_pos(h):
                mo_h = mohs[h]
                kpe_r = kpe_sb[:, mstarts[h]:M].rearrange(
                    "d (p mo) -> d mo p", mo=mo_h
                )
                ps_p = ps_pos_pool.tile([128, maxmo, BPC], F32, tag="ps_p")
                for mo in range(mo_h):
                    nc.tensor.matmul(
                        ps_p[0:64, mo, :], kpe_r[:, mo, 0:64], qts_bf[:, h, :],
                        start=True, stop=True,
                    )
                    nc.tensor.matmul(
                        ps_p[64:128, mo, :], kpe_r[:, mo, 64:128], qts_bf[:, h, :],
                        start=True, stop=True,
                    )
                nc.scalar.copy(pos_sb[h][:], ps_p[:, 0:mo_h, :])

            emit_pos(order[0])

            # ---------------- main loop: heads, then batches ----------------
            for j, h in enumerate(order):
                mo_h = mohs[h]
                mst = mstarts[h]

                if j + 2 < NHEADS:
                    emit_prefetch(order[j + 2])
                if j + 1 < NHEADS:
                    emit_pos(order[j + 1])

                for b in range(BPC):
                    kt, vt = kv_tiles[h][b]
                    if not USE_CAST_DMA:
                        vtb = kv_pool.tile([128, maxmo, HEAD_DIM], BF16, tag="vtb", bufs=2)
                        nc.scalar.copy(vtb[:, 0:mo_h, :], vt[:, 0:mo_h, :])
                        vt = vtb
                    # content scores: scores[p, mo] = sum_d key[..] * q[..]
                    prod = sc_pool.tile([128, maxmo, HEAD_DIM], BF16, tag="prod", bufs=2)
                    q_b = (
                        qrep_bf[:, b, ts(h, HEAD_DIM)]
                        .rearrange("p (x d) -> p x d", x=1)
                        .broadcast_to((128, mo_h, HEAD_DIM))
                    )
                    nc.vector.tensor_mul(prod[:, 0:mo_h, :], kt[:, 0:mo_h, :], q_b)
                    scores = sc_pool.tile([128, maxmo], F32, tag="scores")
                    nc.vector.reduce_sum(
                        scores[:, 0:mo_h], prod[:, 0:mo_h, :], axis=mybir.AxisListType.X
                    )
                    nc.vector.tensor_add(
                        scores[:, 0:mo_h], scores[:, 0:mo_h], pos_sb[h][:, :, b]
                    )
                    # e = exp(scores / sqrt(d)) on ACT, then the mask multiply
                    # on DVE (masks are ready at ~5us, so this op can never
                    # stall the vector stream), with Sigma_w via reduce.
                    # (The reference's extra 1e-8*Sigma_e denominator term is
                    # ~5e-6 relative and is dropped.)
                    e_t = sc_pool.tile([128, maxmo], F32, tag="e_t")
                    nc.scalar.activation(
                        out=e_t[:, 0:mo_h], in_=scores[:, 0:mo_h], func=AF.Exp,
                        scale=float(1.0 / math.sqrt(HEAD_DIM)),
                    )
                    sums = sc_pool.tile([128, 1], F32, tag="sums")
                    w_t = sc_pool.tile([128, maxmo], BF16, tag="w_t")
                    nc.vector.tensor_mul(w_t[:, 0:mo_h], e_t[:, 0:mo_h], masks[h][:])
                    nc.vector.reduce_sum(
                        sums[:, 0:1], w_t[:, 0:mo_h], axis=mybir.AxisListType.X
                    )
                    # partition-reduce Sigma_w, then scal = 1/Sigma_w
                    ps_s = ps_s_pool.tile([1, 1], F32, tag="ps_s")
                    nc.tensor.matmul(
                        ps_s[:], ones_col[:], sums[:], start=True, stop=True
                    )
                    scal = sc_pool.tile([1, 1], F32, tag="scal")
                    nc.vector.reciprocal(scal[:], ps_s[:])
                    # out_row = sum_m w[m] * value[m, :]   (bf16 PE, PSUM accum)
                    ps_o = ps_o_pool.tile([1, HEAD_DIM], F32, tag="ps_o")
                    for mo in range(mo_h):
                        nc.tensor.matmul(
                            ps_o[:],
                            w_t[:, mo : mo + 1],
                            vt[:, mo, :],
                            start=(mo == 0),
                            stop=(mo == mo_h - 1),
                        )
                    # ao[0, b, h*64:(h+1)*64] = ps_o * scal
                    nc.scalar.activation(
                        out=ao_sb[0:1, b, ts(h, HEAD_DIM)], in_=ps_o[:],
                        func=AF.Copy, scale=scal[:, 0:1],
                    )

            # ---------------- output projection -------------------------
            with tc.tile_pool(name="ps_fin", bufs=1, space="PSUM") as ps_fin_pool:
                aoT = []
                for co in range(4):
                    ps_t2 = ps_fin_pool.tile([128, BPC], F32, name="ps_t2", tag="ps_fin")
                    for b in range(BPC):
                        nc.tensor.matmul(
                            ps_t2[:, b : b + 1],
                            ao_sb[0:1, b, ts(co, 128)],
                            identity[0:1, 0:1],
                            start=True, stop=True,
                        )
                    t_sb = fin_pool.tile([128, BPC], BF16, name=f"t_sb{co}", tag=f"t_sb{co}")
                    nc.scalar.copy(t_sb[:], ps_t2[:])
                    aoT.append(t_sb)
                ps_f = ps_fin_pool.tile([BPC, HID], F32, name="ps_f", tag="ps_fin")
                for co in range(4):
                    nc.tensor.matmul(
                        ps_f[:], aoT[co][:], woT[co][:],
                        start=(co == 0), stop=(co == 3),
                    )
                out_sb = fin_pool.tile([BPC, HID], F32, tag="out_sb")
                nc.scalar.copy(out_sb[:], ps_f[:])
                nc.sync.dma_start(out=out_d[:], in_=out_sb[:])

    nc.compile()
    return nc


def _get_nc(mstarts):
    if mstarts not in _CACHE:
        _CACHE[mstarts] = build_nc(mstarts)
    return _CACHE[mstarts]


def _make_in_maps(query, key, value, Wq, Wo, key_pe, span):
    q2 = np.ascontiguousarray(np.asarray(query, np.float32).reshape(B, HID))
    key = np.asarray(key, np.float32)
    value = np.asarray(value, np.float32)
    Wq = np.ascontiguousarray(np.asarray(Wq, np.float32))
    Wo = np.ascontiguousarray(np.asarray(Wo, np.float32))
    key_pe = np.ascontiguousarray(np.asarray(key_pe, np.float32))
    span = np.ascontiguousarray(np.asarray(span, np.float32))
    in_maps = []
    for c in range(N_CORES):
        in_maps.append(
            {
                "query": np.ascontiguousarray(q2[c * BPC : (c + 1) * BPC]),
                "key": np.ascontiguousarray(key[c * NPC : (c + 1) * NPC]),
                "value": np.ascontiguousarray(value[c * NPC : (c + 1) * NPC]),
                "Wq": Wq,
                "Wo": Wo,
                "key_pe": key_pe,
                "span": span,
            }
        )
    return in_maps


def _install_ntff_hook():
    """Shim antenv.axon_hooks with a ctypes NTFF profile hook so
    run_bass_kernel_spmd(trace=True) works in this container."""
    import contextlib
    import ctypes
    import types

    try:
        import antenv.axon_hooks  # noqa: F401

        return
    except ImportError:
        pass
    so_path = "/opt/axon/libaxon_pjrt.so"
    import antenv

    mod = types.ModuleType("antenv.axon_hooks")
    holder = {"hook": None}

    if os.path.exists(so_path):
        lib = ctypes.CDLL(so_path)
        if hasattr(lib, "axon_start_nrt_profile"):
            lib.axon_start_nrt_profile.argtypes = [
                ctypes.POINTER(ctypes.c_int64),
                ctypes.c_size_t,
            ]
            lib.axon_start_nrt_profile.restype = ctypes.c_int64
            lib.axon_stop_nrt_profile.argtypes = [ctypes.c_char_p]
            lib.axon_stop_nrt_profile.restype = ctypes.c_int64

            @contextlib.contextmanager
            def _hook(output_dir, device_ids):
                import jax

                jax.devices()
                if device_ids:
                    ids = (ctypes.c_int64 * len(device_ids))(*device_ids)
                    rc = lib.axon_start_nrt_profile(ids, len(device_ids))
                else:
                    rc = lib.axon_start_nrt_profile(None, 0)
                if rc != 0:
                    raise RuntimeError(f"axon_start_nrt_profile rc={rc}")
                try:
                    yield
                finally:
                    n = lib.axon_stop_nrt_profile(str(output_dir).encode())
                    print(f"profile: {n} file(s) written to {output_dir}")

            holder["hook"] = _hook

    mod.get_axon_ntff_profile_hook = lambda: holder["hook"]
    mod.set_axon_ntff_profile_hook = lambda h: holder.__setitem__("hook", h)
    sys.modules["antenv.axon_hooks"] = mod
    antenv.axon_hooks = mod


def run(query, key, value, Wq, Wo, key_pe, span, trace=False):
    """Run on hardware; returns (output [B,1,HID], BassKernelResults)."""
    from concourse import bass_utils
    from concourse.bass_utils import run_bass_kernel_spmd

    if trace:
        _install_ntff_hook()
        bass_utils.upload_artifacts = lambda tmpdir: f"local:{tmpdir}"
    nc = _get_nc(compute_mstarts(span))
    in_maps = _make_in_maps(query, key, value, Wq, Wo, key_pe, span)
    res = run_bass_kernel_spmd(nc, in_maps, list(range(N_CORES)), trace=trace)
    out = np.concatenate(
        [np.asarray(res.results[c]["out"]) for c in range(N_CORES)], axis=0
    )
    return out.reshape(B, 1, HID).astype(np.float32), res


def kernel(query, key, value, Wq, Wo, key_pe, span):
    out, _ = run(query, key, value, Wq, Wo, key_pe, span, trace=False)
    return out


# revision 40
# speedup vs baseline: 1.1842x; 1.0875x over previous
"""Trainium2 Bass kernel for multi-head attention with adaptive span masking.

Computation (per the nn.Module):
    q = (query @ Wq.T) split into B*H rows of size d=64
    attn = softmax((key . q + q @ key_pe) / sqrt(d))
    attn = renormalize(attn * adaptive_span_mask)
    out = (attn . value) merged heads @ Wo.T

Key optimization: the adaptive-span mask is exactly zero for positions
m <= (M-1) - RAMP - span[h]*M, so those key/value rows contribute
nothing to the output (their only coupling is the 1e-8*sum(softmax)
term in the renormalization denominator, which perturbs the result by
~5e-6 relative). Each head therefore only reads the tail [mstart_h, M)
of key/value, cutting HBM traffic by ~2x. mstart_h is computed on the
host from the actual span input and baked into the compiled kernel.

Sharding: batch-parallel across 8 cores. Core c gets batches [4c, 4c+4)
(all 8 heads) = rows [32c, 32c+32) of key/value; Wq/Wo/key_pe/span are
replicated. Each core produces its own [4, 512] output block; the host
concatenates. No collectives needed.

Engine split per (batch, head) row:
  - key AND value loaded via gpsimd (SWDGE) DMA with inline f32->bf16
    cast: HBM reads stay f32 (unavoidable) but no engine time is spent
    casting and SBUF footprint halves
  - QK dot on DVE: bf16 multiply + reduce over d
  - positional scores precomputed per head on PE (key_pe stationary)
  - exp on ACT (with fused sum); mask-mult + sum fused in one DVE
    tensor_tensor_reduce; PV accumulation on PE in bf16
  - prefetch is software-pipelined two heads ahead so the DMA queues
    never drain; masks/iotas are computed in setup so the gpsimd queue
    carries only DMA work in the main loop
"""

import math
import os
import sys

import numpy as np

for _p in ("/opt/trn_rl_repo", "/root/.axon_site/_ro/trn_rl_repo"):
    if os.path.isdir(_p) and _p not in sys.path:
        sys.path.insert(0, _p)

import concourse.bass as bass
import concourse.bacc as bacc
import concourse.mybir as mybir
from concourse.bass import ts
from concourse.masks import make_identity
from concourse.tile import TileContext

F32 = mybir.dt.float32
BF16 = mybir.dt.bfloat16

# Problem constants (hardcoded per contest contract)
NHEADS = 8
HEAD_DIM = 64
HID = NHEADS * HEAD_DIM  # 512
B = 32
M = 8192
RAMP = 32.0

N_CORES = 8
BPC = B // N_CORES        # 4 batches per core
NPC = BPC * NHEADS        # 32 (b,h) rows per core

# tensor_tensor_reduce faults the runtime on this deployment; keep the
# two-op mul+reduce path (opt back in with K_TTR=1 to re-test)
USE_TTR = os.environ.get("K_TTR", "") != ""
USE_CAST_DMA = os.environ.get("K_NO_CAST", "") == ""

_CACHE = {}


def compute_mstarts(span: np.ndarray) -> tuple:
    """First key/value position with nonzero mask, per head, aligned
    down to a multiple of 128 (the SBUF partition count).

    mask[h, m] = clip((m - (M-1) + span[h]*M)/RAMP + 1, 0, 1) is zero
    iff m <= (M-1) - RAMP - span[h]*M.
    """
    s = np.asarray(span, np.float64).reshape(-1)
    last_zero = np.floor((M - 1) - RAMP - s * M).astype(np.int64)
    mstart = np.clip(last_zero, 0, M - 128)
    mstart = (mstart // 128) * 128
    return tuple(int(x) for x in mstart)


def build_nc(mstarts):
    nc = bacc.Bacc(None, target_bir_lowering=False)
    AF = mybir.ActivationFunctionType
    ALU = mybir.AluOpType

    mohs = [(M - mstarts[h]) // 128 for h in range(NHEADS)]
    maxmo = max(mohs)
    # big heads first (their DMA overlaps setup), smallest last (short tail)
    order = sorted(range(NHEADS), key=lambda h: -mohs[h])

    q_d = nc.dram_tensor("query", [BPC, HID], F32, kind="ExternalInput")
    k_d = nc.dram_tensor("key", [NPC, M, HEAD_DIM], F32, kind="ExternalInput")
    v_d = nc.dram_tensor("value", [NPC, M, HEAD_DIM], F32, kind="ExternalInput")
    wq_d = nc.dram_tensor("Wq", [HID, HID], F32, kind="ExternalInput")
    wo_d = nc.dram_tensor("Wo", [HID, HID], F32, kind="ExternalInput")
    kpe_d = nc.dram_tensor("key_pe", [HEAD_DIM, M], F32, kind="ExternalInput")
    span_d = nc.dram_tensor("span", [NHEADS, 1], F32, kind="ExternalInput")
    out_d = nc.dram_tensor("out", [BPC, HID], F32, kind="ExternalOutput")

    with TileContext(nc) as tc:
        with (
            tc.tile_pool(name="persist", bufs=1) as persist,
            # main-loop pools created BEFORE setup pools so the kv DMAs get
            # SBUF ranges disjoint from setup tiles (no WAR dep -> kv loads
            # start at t=0, overlapping the whole setup phase)
            tc.tile_pool(name="kv", bufs=8) as kv_pool,
            tc.tile_pool(name="sc", bufs=3) as sc_pool,
            tc.tile_pool(name="fin", bufs=1) as fin_pool,
            tc.tile_pool(name="ps_pos", bufs=2, space="PSUM") as ps_pos_pool,
            tc.tile_pool(name="ps_s", bufs=1, space="PSUM") as ps_s_pool,
            tc.tile_pool(name="ps_o", bufs=2, space="PSUM") as ps_o_pool,
        ):
            identity = persist.tile([128, 128], F32, tag="identity")
            make_identity(nc, identity[:])
            identity_bf = persist.tile([128, 128], BF16, tag="identity_bf")
            nc.vector.tensor_scalar(
                out=identity_bf[:], in0=identity[:],
                scalar1=1.0, scalar2=0.0, op0=ALU.mult, op1=ALU.add,
            )
            ones_row = persist.tile([1, 128], F32, tag="ones_row")
            nc.vector.memset(ones_row[:], 1.0)
            ones_col = persist.tile([128, 1], F32, tag="ones_col")
            nc.vector.memset(ones_col[:], 1.0)

            woT = [persist.tile([128, HID], BF16, name=f"woT{j}", tag=f"woT{j}") for j in range(4)]
            q_sb = persist.tile([BPC, HID], F32, tag="q_sb")
            qts = persist.tile([HEAD_DIM, NHEADS, BPC], F32, tag="qts")
            qts_bf = persist.tile([HEAD_DIM, NHEADS, BPC], BF16, tag="qts_bf")
            qrep_bf = persist.tile([128, BPC, HID], BF16, tag="qrep_bf")
            kpe_sb = persist.tile([HEAD_DIM, M], BF16, tag="kpe_sb")
            span_b = persist.tile([128, NHEADS], F32, tag="span_b")
            span_row = persist.tile([1, NHEADS], F32, tag="span_row")
            pos_sb = [
                persist.tile([128, mohs[h], BPC], F32, name=f"pos{h}", tag=f"pos{h}")
                for h in range(NHEADS)
            ]
            masks = [
                persist.tile([128, mohs[h]], F32, name=f"mask{h}", tag=f"mask{h}")
                for h in range(NHEADS)
            ]
            ao_sb = persist.tile([1, BPC, HID], F32, tag="ao_sb")

            # key_pe tail (only columns any head can touch), cast to bf16
            # in-flight; first instruction on the gpsimd queue. The weight
            # matrices and query also come in as bf16 cast-DMAs: their
            # transposes then run as cheap single-pass bf16 matmuls instead
            # of multi-pass fp32 ones, collapsing the PE front-end that every
            # downstream chain waits on.
            mstart_min = min(mstarts)
            nc.gpsimd.dma_start(
                out=kpe_sb[:, mstart_min:M], in_=kpe_d[:, mstart_min:M]
            )
            wq_bf = [persist.tile([128, HID], BF16, name=f"wq_bf{i}", tag=f"wq_bf{i}") for i in range(4)]
            wo_bf = [persist.tile([128, HID], BF16, name=f"wo_bf{i}", tag=f"wo_bf{i}") for i in range(4)]
            query_bf = persist.tile([BPC, HID], BF16, tag="query_bf")
            for i in range(4):
                nc.gpsimd.dma_start(out=wq_bf[i][:], in_=wq_d[ts(i, 128), :])
            nc.gpsimd.dma_start(out=query_bf[:], in_=q_d[:])
            for i in range(4):
                nc.gpsimd.dma_start(out=wo_bf[i][:], in_=wo_d[ts(i, 128), :])

            # ---- span bias + adaptive-span masks, computed FIRST so the
            # row pipeline is never gated on them (they feed every row's
            # mask multiply)
            nc.sync.dma_start(out=span_row[:], in_=span_d[:].rearrange("h o -> o h"))
            span_b2 = persist.tile([128, NHEADS], F32, tag="span_b2")
            with tc.tile_pool(name="ps_span", bufs=1, space="PSUM") as ps_span:
                ps_sp = ps_span.tile([128, NHEADS], F32, tag="ps_sp")
                nc.tensor.matmul(
                    ps_sp[:], ones_row[:], span_row[:], start=True, stop=True
                )
                # span_b[p, h] = span[h]*M/RAMP - (M-1)/RAMP + 1;
                # span_b2 folds in the per-head mstart/RAMP offset
                bias_const = float(-(M - 1) / RAMP + 1.0)
                nc.scalar.activation(
                    out=span_b[:], in_=ps_sp[:], func=AF.Copy,
                    scale=float(M / RAMP), bias=bias_const,
                )
            for h in range(NHEADS):
                nc.scalar.activation(
                    out=span_b2[:, h : h + 1], in_=span_b[:, h : h + 1],
                    func=AF.Copy, bias=float(mstarts[h] / RAMP),
                )
            # masks[h][p, j] = clip((mstart_h + p*mo_h + j)/RAMP
            #                       + span_b[h], 0, 1)
            # (applied per row as a cheap [128, mo_h] multiply whose only
            # dependency is this early block -- keeping the vector stream's
            # early ops shallow so the scheduler cannot stall it)
            for h in range(NHEADS):
                mo_h = mohs[h]
                m_f = sc_pool.tile([128, maxmo], F32, tag="m_f", bufs=4)
                nc.gpsimd.iota(
                    out=m_f[:, 0:mo_h], pattern=[[1, mo_h]], base=0,
                    channel_multiplier=mo_h,
                    allow_small_or_imprecise_dtypes=True,
                )
                nc.scalar.activation(
                    out=masks[h][:], in_=m_f[:, 0:mo_h], func=AF.Identity,
                    scale=float(1.0 / RAMP), bias=span_b2[:, h : h + 1],
                )
                nc.vector.tensor_scalar(
                    out=masks[h][:], in0=masks[h][:],
                    scalar1=0.0, scalar2=1.0,
                    op0=ALU.max, op1=ALU.min,
                )

            # K/V prefetch for one head (4 batch rows), f32->bf16 in-flight
            kv_tiles = {}

            def emit_prefetch(h):
                # one batched cast-DMA per tensor per head: all 4 batch rows
                # in a single transfer (4x bigger than per-row -> better SDMA
                # efficiency for the small heads)
                mo_h = mohs[h]
                mst = mstarts[h]
                k_r = (
                    k_d[:]
                    .rearrange("(b hh) m d -> hh b m d", hh=NHEADS)[h, :, mst:M, :]
                    .rearrange("b (p mo) d -> p b mo d", p=128)
                )
                v_r = (
                    v_d[:]
                    .rearrange("(b hh) m d -> hh b m d", hh=NHEADS)[h, :, mst:M, :]
                    .rearrange("b (p mo) d -> p b mo d", p=128)
                )
                kt = kv_pool.tile([128, BPC, maxmo, HEAD_DIM], BF16, tag="kt", bufs=2)
                nc.gpsimd.dma_start(out=kt[:, :, 0:mo_h, :], in_=k_r)
                vt = kv_pool.tile([128, BPC, maxmo, HEAD_DIM], BF16, tag="vt", bufs=2)
                nc.gpsimd.dma_start(out=vt[:, :, 0:mo_h, :], in_=v_r)
                kv_tiles[h] = (kt, vt)

            emit_prefetch(order[0])
            emit_prefetch(order[1])

            # ---------------- setup phase A: weight transposes + q ----------
            with (
                tc.tile_pool(name="setupA", bufs=1) as sa,
                tc.tile_pool(name="psA", bufs=2, space="PSUM") as psA,
            ):
                # all transposes as single-pass bf16 matmuls with 64-column
                # stationary halves (no fp32 multi-pass, no 128-col FWL)
                def transpose_blk(dst_slice, src_blk, pool):
                    # src_blk [128, 128] bf16 -> dst [128, 128] = src^T
                    pwt = pool.tile([128, 128], F32, tag="pwt")
                    nc.tensor.matmul(
                        pwt[0:64, :], src_blk[:, 0:64], identity_bf[:],
                        start=True, stop=True,
                    )
                    nc.tensor.matmul(
                        pwt[64:128, :], src_blk[:, 64:128], identity_bf[:],
                        start=True, stop=True,
                    )
                    nc.scalar.copy(dst_slice, pwt[:])

                wqT = [sa.tile([128, HID], BF16, name=f"wqT{j}", tag=f"wqT{j}") for j in range(4)]
                for io in range(4):
                    for jo in range(4):
                        transpose_blk(
                            wqT[jo][:, ts(io, 128)],
                            wq_bf[io][:, ts(jo, 128)], psA,
                        )
                qTq = [sa.tile([128, BPC], BF16, name=f"qTq{j}", tag=f"qTq{j}") for j in range(4)]
                for jo in range(4):
                    pqt = psA.tile([128, BPC], F32, tag="pwt")
                    nc.tensor.matmul(
                        pqt[0:64, :], query_bf[:, ts(2 * jo, 64)],
                        identity_bf[0:BPC, 0:BPC],
                        start=True, stop=True,
                    )
                    nc.tensor.matmul(
                        pqt[64:128, :], query_bf[:, ts(2 * jo + 1, 64)],
                        identity_bf[0:BPC, 0:BPC],
                        start=True, stop=True,
                    )
                    nc.scalar.copy(qTq[jo][:], pqt[:])
                # q = query @ Wq.T  ->  [4, 512]
                ps_q = psA.tile([BPC, HID], F32, tag="ps_q", bufs=1)
                for jo in range(4):
                    nc.tensor.matmul(
                        ps_q[:], qTq[jo][:], wqT[jo][:],
                        start=(jo == 0), stop=(jo == 3),
                    )
                nc.scalar.copy(q_sb[:], ps_q[:])
                # qts[d, h, b] = q[b, h*64+d]   (64 partitions)
                for h in range(NHEADS):
                    pqh = psA.tile([HEAD_DIM, BPC], F32, tag="pwt")
                    nc.tensor.matmul(
                        pqh[:], q_sb[:, ts(h, HEAD_DIM)], identity[0:BPC, 0:BPC],
                        start=True, stop=True,
                    )
                    nc.scalar.copy(qts[:, h, :], pqh[:])
                nc.scalar.copy(qts_bf[:], qts[:])

                # Wo transposes (bf16, only consumed by the final projection)
                for io in range(4):
                    for jo in range(4):
                        transpose_blk(
                            woT[jo][:, ts(io, 128)],
                            wo_bf[io][:, ts(jo, 128)], psA,
                        )

            # ---------------- setup phase B: qrep broadcast -----------------
            # qrep_bf[p, b, :] = q[b, :] on every partition: bounce q via
            # DRAM, then ONE sync-queue (HWDGE) broadcast DMA of the flat
            # [1, 2048] row to all 128 partitions + a single ACT cast. The
            # sync queue is free of the K/V flood, so this is ready early.
            qrep = persist.tile([128, BPC * HID], F32, tag="qrep")
            with tc.tile_pool(name="dramq", bufs=1, space="DRAM") as dq:
                q_dram = dq.tile([BPC, HID], F32, tag="q_dram")
                nc.sync.dma_start(out=q_dram[:], in_=q_sb[:])
                nc.sync.dma_start(
                    out=qrep[:],
                    in_=q_dram[:].rearrange("b f -> (b f)").partition_broadcast(128),
                )
            # cast on the DVE: the scalar engine's stream fills up with
            # PSUM->SBUF pos copies and the list scheduler pushed this cast
            # behind all of them, gating every row multiply until ~96us
            nc.vector.tensor_scalar(
                out=qrep_bf[:], in0=qrep[:].rearrange("p (b f) -> p b f", b=BPC),
                scalar1=1.0, scalar2=0.0, op0=ALU.mult, op1=ALU.add,
            )

            # positional scores for one head, software-pipelined one head
            # ahead of the row compute:
            # pos[p, mo, b] = sum_d key_pe[d, m] * q[b, h*64+d]
            # The stationary kpe slice is strided, so it is loaded as two
            # 64-column halves: a 128-column non-f32 stationary would engage
            # the compiler's fast-weight-load path, which assumes contiguous
            # weights.
            def emit_pos(h):
                mo_h = mohs[h]
                kpe_r = kpe_sb[:, mstarts[h]:M].rearrange(
                    "d (p mo) -> d mo p", mo=mo_h
                )
                ps_p = ps_pos_pool.tile([128, maxmo, BPC], F32, tag="ps_p")
                for mo in range(mo_h):
                    nc.tensor.matmul(
                        ps_p[0:64, mo, :], kpe_r[:, mo, 0:64], qts_bf[:, h, :],
                        start=True, stop=True,
                    )
                    nc.tensor.matmul(
                        ps_p[64:128, mo, :], kpe_r[:, mo, 64:128], qts_bf[:, h, :],
                        start=True, stop=True,
                    )
                nc.scalar.copy(pos_sb[h][:], ps_p[:, 0:mo_h, :])

            emit_pos(order[0])

            # ---------------- main loop: heads, then batches ----------------
            for j, h in enumerate(order):
                mo_h = mohs[h]
                mst = mstarts[h]

                if j + 2 < NHEADS:
                    emit_prefetch(order[j + 2])
                if j + 1 < NHEADS:
                    emit_pos(order[j + 1])

                kt_h, vt_h = kv_tiles[h]
                for b in range(BPC):
                    kt = kt_h[:, b]
                    vt = vt_h[:, b]
                    # content scores: scores[p, mo] = sum_d key[..] * q[..]
                    prod = sc_pool.tile([128, maxmo, HEAD_DIM], BF16, tag="prod", bufs=3)
                    q_b = (
                        qrep_bf[:, b, ts(h, HEAD_DIM)]
                        .rearrange("p (x d) -> p x d", x=1)
                        .broadcast_to((128, mo_h, HEAD_DIM))
                    )
                    nc.vector.tensor_mul(prod[:, 0:mo_h, :], kt[:, 0:mo_h, :], q_b)
                    scores = sc_pool.tile([128, maxmo], F32, tag="scores")
                    nc.vector.reduce_sum(
                        scores[:, 0:mo_h], prod[:, 0:mo_h, :], axis=mybir.AxisListType.X
                    )
                    nc.vector.tensor_add(
                        scores[:, 0:mo_h], scores[:, 0:mo_h], pos_sb[h][:, :, b]
                    )
                    # e = exp(scores / sqrt(d)) on ACT, then the mask multiply
                    # on DVE (masks are ready at ~5us, so this op can never
                    # stall the vector stream), with Sigma_w via reduce.
                    # (The reference's extra 1e-8*Sigma_e denominator term is
                    # ~5e-6 relative and is dropped.)
                    e_t = sc_pool.tile([128, maxmo], F32, tag="e_t")
                    nc.scalar.activation(
                        out=e_t[:, 0:mo_h], in_=scores[:, 0:mo_h], func=AF.Exp,
                        scale=float(1.0 / math.sqrt(HEAD_DIM)),
                    )
                    sums = sc_pool.tile([128, 1], F32, tag="sums")
                    w_t = sc_pool.tile([128, maxmo], BF16, tag="w_t")
                    nc.vector.tensor_mul(w_t[:, 0:mo_h], e_t[:, 0:mo_h], masks[h][:])
                    nc.vector.reduce_sum(
                        sums[:, 0:1], w_t[:, 0:mo_h], axis=mybir.AxisListType.X
                    )
                    # partition-reduce Sigma_w, then scal = 1/Sigma_w
                    ps_s = ps_s_pool.tile([1, 1], F32, tag="ps_s")
                    nc.tensor.matmul(
                        ps_s[:], ones_col[:], sums[:], start=True, stop=True
                    )
                    scal = sc_pool.tile([1, 1], F32, tag="scal")
                    nc.vector.reciprocal(scal[:], ps_s[:])
                    # out_row = sum_m w[m] * value[m, :]   (bf16 PE, PSUM accum)
                    ps_o = ps_o_pool.tile([1, HEAD_DIM], F32, tag="ps_o")
                    for mo in range(mo_h):
                        nc.tensor.matmul(
                            ps_o[:],
                            w_t[:, mo : mo + 1],
                            vt[:, mo, :],
                            start=(mo == 0),
                            stop=(mo == mo_h - 1),
                        )
                    # ao[0, b, h*64:(h+1)*64] = ps_o * scal
                    nc.scalar.activation(
                        out=ao_sb[0:1, b, ts(h, HEAD_DIM)], in_=ps_o[:],
                        func=AF.Copy, scale=scal[:, 0:1],
                    )

            # ---------------- output projection -------------------------
            with tc.tile_pool(name="ps_fin", bufs=1, space="PSUM") as ps_fin_pool:
                aoT = []
                for co in range(4):
                    ps_t2 = ps_fin_pool.tile([128, BPC], F32, name="ps_t2", tag="ps_fin")
                    for b in range(BPC):
                        nc.tensor.matmul(
                            ps_t2[:, b : b + 1],
                            ao_sb[0:1, b, ts(co, 128)],
                            identity[0:1, 0:1],
                            start=True, stop=True,
                        )
                    t_sb = fin_pool.tile([128, BPC], BF16, name=f"t_sb{co}", tag=f"t_sb{co}")
                    nc.scalar.copy(t_sb[:], ps_t2[:])
                    aoT.append(t_sb)
                ps_f = ps_fin_pool.tile([BPC, HID], F32, name="ps_f", tag="ps_fin")
                for co in range(4):
                    nc.tensor.matmul(
                        ps_f[:], aoT[co][:], woT[co][:],
                        start=(co == 0), stop=(co == 3),
                    )
                out_sb = fin_pool.tile([BPC, HID], F32, tag="out_sb")
                nc.scalar.copy(out_sb[:], ps_f[:])
                nc.sync.dma_start(out=out_d[:], in_=out_sb[:])

    nc.compile()
    return nc


def _get_nc(mstarts):
    if mstarts not in _CACHE:
        _CACHE[mstarts] = build_nc(mstarts)
    return _CACHE[mstarts]


def _make_in_maps(query, key, value, Wq, Wo, key_pe, span):
    q2 = np.ascontiguousarray(np.asarray(query, np.float32).reshape(B, HID))
    key = np.asarray(key, np.float32)
    value = np.asarray(value, np.float32)
    Wq = np.ascontiguousarray(np.asarray(Wq, np.float32))
    Wo = np.ascontiguousarray(np.asarray(Wo, np.float32))
    key_pe = np.ascontiguousarray(np.asarray(key_pe, np.float32))
    span = np.ascontiguousarray(np.asarray(span, np.float32))
    in_maps = []
    for c in range(N_CORES):
        in_maps.append(
            {
                "query": np.ascontiguousarray(q2[c * BPC : (c + 1) * BPC]),
                "key": np.ascontiguousarray(key[c * NPC : (c + 1) * NPC]),
                "value": np.ascontiguousarray(value[c * NPC : (c + 1) * NPC]),
                "Wq": Wq,
                "Wo": Wo,
                "key_pe": key_pe,
                "span": span,
            }
        )
    return in_maps


def _install_ntff_hook():
    """Shim antenv.axon_hooks with a ctypes NTFF profile hook so
    run_bass_kernel_spmd(trace=True) works in this container."""
    import contextlib
    import ctypes
    import types

    try:
        import antenv.axon_hooks  # noqa: F401

        return
    except ImportError:
        pass
    so_path = "/opt/axon/libaxon_pjrt.so"
    import antenv

    mod = types.ModuleType("antenv.axon_hooks")
    holder = {"hook": None}

    if os.path.exists(so_path):
        lib = ctypes.CDLL(so_path)
        if hasattr(lib, "axon_start_nrt_profile"):
            lib.axon_start_nrt_profile.argtypes = [
                ctypes.POINTER(ctypes.c_int64),
                ctypes.c_size_t,
            ]
            lib.axon_start_nrt_profile.restype = ctypes.c_int64
            lib.axon_stop_nrt_profile.argtypes = [ctypes.c_char_p]
            lib.axon_stop_nrt_profile.restype = ctypes.c_int64

            @contextlib.contextmanager
            def _hook(output_dir, device_ids):
                import jax

                jax.devices()
                if device_ids:
                    ids = (ctypes.c_int64 * len(device_ids))(*device_ids)
                    rc = lib.axon_start_nrt_profile(ids, len(device_ids))
                else:
                    rc = lib.axon_start_nrt_profile(None, 0)
                if rc != 0:
                    raise RuntimeError(f"axon_start_nrt_profile rc={rc}")
                try:
                    yield
                finally:
                    n = lib.axon_stop_nrt_profile(str(output_dir).encode())
                    print(f"profile: {n} file(s) written to {output_dir}")

            holder["hook"] = _hook

    mod.get_axon_ntff_profile_hook = lambda: holder["hook"]
    mod.set_axon_ntff_profile_hook = lambda h: holder.__setitem__("hook", h)
    sys.modules["antenv.axon_hooks"] = mod
    antenv.axon_hooks = mod


def run(query, key, value, Wq, Wo, key_pe, span, trace=False):
    """Run on hardware; returns (output [B,1,HID], BassKernelResults)."""
    from concourse import bass_utils
    from concourse.bass_utils import run_bass_kernel_spmd

    if trace:
        _install_ntff_hook()
        bass_utils.upload_artifacts = lambda tmpdir: f"local:{tmpdir}"
    nc = _get_nc(compute_mstarts(span))
    in_maps = _make_in_maps(query, key, value, Wq, Wo, key_pe, span)
    res = run_bass_kernel_spmd(nc, in_maps, list(range(N_CORES)), trace=trace)
    out = np.concatenate(
        [np.asarray(res.results[c]["out"]) for c in range(N_CORES)], axis=0
    )
    return out.reshape(B, 1, HID).astype(np.float32), res


def kernel(query, key, value, Wq, Wo, key_pe, span):
    out, _ = run(query, key, value, Wq, Wo, key_pe, span, trace=False)
    return out
